# revision 27
# baseline (speedup 1.0000x reference)
"""Trainium2 Bass kernel for nn_AdvancedSFIN (hierarchical complex transformer).

Self-contained: builds a single-core Bass/Tile program, runs it SPMD on 8
NeuronCores (data-parallel over batch: 2 sequences per core), reassembles the
full [16, 256, 32000] float32 output on the host.

Design notes:
- Activations live in "layout B": [D(partitions, 4 chunks of 128), tok(free)].
  Linear layers contract over D natively (lhsT = host-pre-transposed weights).
- All matmuls run in bf16 (validated ~3.4e-3 scale-relative error end to end);
  LayerNorm statistics matmuls use float32r (full-rate fp32 path).
- LayerNorm over D (partition axis) uses ones-vector matmuls for sum/sum-sq,
  gpsimd partition_broadcast for the per-token stats, and exp(-0.5*ln(var+eps))
  on the Scalar engine for rsqrt (keeps the natural_log_exp ACT table resident;
  sqrt never appears on ACT).
- Attention per (seq, head): scoresT = k_cat.T @ q_cat with [real|imag]
  concatenated on the contraction axis; softmax over the partition (ki) axis
  via ones-matmul sums (scores are bounded ~|1.7| so no max subtraction);
  AV uses token-major V (produced directly by using the normalized input as
  the stationary matmul operand), real/imag packed into one PSUM tile by
  column tiling; the 1/sum normalization rides on PSUM eviction.
- ln_hier gamma/beta are folded into the QKV weights on the host (the cln
  ahead of each attention feeds only those linears).
- Memory read + collapse produce logits v-major [V, tok] so the collapse bias
  + relu(+1e-10 folded into the bias) ride per-partition on ScalarE; the host
  transposes back and upcasts bf16 -> f32.
"""

import numpy as np
import ml_dtypes
from contextlib import ExitStack

import concourse.bass as bass
import concourse.bacc as bacc
import concourse.tile as tile
from concourse import mybir
from concourse.bass_utils import run_bass_kernel_spmd
from concourse.masks import make_identity

P = 128
V, D, H, M_MEM = 32000, 512, 8, 512
B, S = 16, 256
DEPTH = 2
NCORES = 8
BPC = B // NCORES          # sequences per core
TOK = BPC * S              # tokens per core (512)
HD = D // H                # head dim (64)
NCH = D // P               # d-chunks (4)
NVT = V // P               # collapse v-tiles (250)
EPS = 1e-5

F32 = mybir.dt.float32
F32R = mybir.dt.float32r
BF16 = mybir.dt.bfloat16
I32 = mybir.dt.int32
AF = mybir.ActivationFunctionType
OP = mybir.AluOpType
AX = mybir.AxisListType

BF = ml_dtypes.bfloat16

DEBUG = False  # when True, adds DRAM taps for intermediates (dev only)

LAYERS = [f"w{i}" for i in range(DEPTH)] + \
         [f"p{i}" for i in range(DEPTH)] + \
         [f"s{i}" for i in range(max(1, DEPTH // 2))]


# ----------------------------------------------------------------------------
# host-side prep
# ----------------------------------------------------------------------------

def _wt_tiled(w_t: np.ndarray) -> np.ndarray:
    """[din, dout] -> [128, din//128, dout] bf16 (k-chunk c at [:, c, :])."""
    din, dout = w_t.shape
    return np.ascontiguousarray(
        w_t.reshape(din // P, P, dout).transpose(1, 0, 2)).astype(BF)


def _bias_pp(b: np.ndarray) -> np.ndarray:
    """[512] -> [128, 4] f32 per-partition layout (chunk c in column c)."""
    return np.ascontiguousarray(b.reshape(-1, P).T).astype(np.float32)


def _prep(x: np.ndarray, params: dict):
    gh = np.asarray(params["ln_hier_g"], np.float32)
    bh = np.asarray(params["ln_hier_b"], np.float32)

    shared = {}
    shared["emb_real"] = np.ascontiguousarray(np.asarray(params["emb_real"], np.float32))
    shared["emb_imag"] = np.ascontiguousarray(np.asarray(params["emb_imag"], np.float32))
    shared["freq"] = np.ascontiguousarray(
        np.asarray(params["freq"], np.float32).reshape(1, D))
    sr = np.arange(S, dtype=np.float32).reshape(S // P, P).T
    shared["srange"] = np.ascontiguousarray(sr)

    def attn_prep(prefix, p):
        for wname in ("q", "k"):
            w = np.asarray(p[wname]["W"], np.float32)
            b = np.asarray(p[wname]["b"], np.float32)
            w_eff = w * gh[None, :]
            b_eff = b + w @ bh
            shared[f"{prefix}_{wname}T"] = _wt_tiled(w_eff.T)
            bb = np.empty((P, H), np.float32)
            for h in range(H):
                bb[0:64, h] = b_eff[64 * h:64 * h + 64]
                bb[64:128, h] = b_eff[64 * h:64 * h + 64]
            shared[f"{prefix}_{wname}b"] = np.ascontiguousarray(bb)
        w = np.asarray(p["v"]["W"], np.float32)
        b = np.asarray(p["v"]["b"], np.float32)
        shared[f"{prefix}_vT"] = _wt_tiled((w * gh[None, :]).T)
        shared[f"{prefix}_vbrow"] = np.ascontiguousarray(
            (b + w @ bh).reshape(1, D)).astype(np.float32)
        w = np.asarray(p["o"]["W"], np.float32)
        b = np.asarray(p["o"]["b"], np.float32)
        shared[f"{prefix}_oT"] = _wt_tiled(w.T)
        shared[f"{prefix}_ob"] = _bias_pp(b)

    for prefix, p in zip(LAYERS, list(params["word"]) + list(params["phrase"])
                         + list(params["sentence"])):
        attn_prep(prefix, p)

    for nm, key in (("wp", "proj_wp"), ("ps", "proj_ps"), ("rd", "read")):
        w = np.asarray(params[key]["W"], np.float32)
        b = np.asarray(params[key]["b"], np.float32)
        shared[f"{nm}T"] = _wt_tiled(w.T)
        shared[f"{nm}b"] = _bias_pp(b)

    mem_r = np.asarray(params["mem_real"], np.float32)
    mem_i = np.asarray(params["mem_imag"], np.float32)
    shared["memT"] = _wt_tiled(mem_r.T)
    shared["mem_r"] = _wt_tiled(mem_r)
    shared["mem_i"] = _wt_tiled(mem_i)

    shared["ln_model_g"] = _bias_pp(np.asarray(params["ln_model_g"], np.float32))
    shared["ln_model_b"] = _bias_pp(np.asarray(params["ln_model_b"], np.float32))
    shared["ln_hier_g"] = _bias_pp(gh)
    shared["ln_hier_b"] = _bias_pp(bh)

    cw = np.asarray(params["collapse_W"], np.float32)   # [V, D]
    cb = np.asarray(params["collapse_b"], np.float32)
    A = cw.reshape(NVT, P, NCH, P).transpose(3, 0, 2, 1)   # [p, t, c, v']
    shared["cwT"] = np.ascontiguousarray(A.reshape(P, NVT, D)).astype(BF)
    shared["cb"] = np.ascontiguousarray(
        (cb + 1e-10).reshape(NVT, P).T).astype(np.float32)

    x = np.asarray(x).astype(np.int32)
    per_core = []
    for c in range(NCORES):
        m = dict(shared)
        m["xidx"] = np.ascontiguousarray(x[BPC * c:BPC * (c + 1)].reshape(TOK, 1))
        per_core.append(m)
    return per_core


# ----------------------------------------------------------------------------
# device program
# ----------------------------------------------------------------------------

class K:
    def __init__(self, ctx, tc):
        self.ctx = ctx
        self.tc = tc
        self.nc = tc.nc
        nc = self.nc
        ep = ctx.enter_context

        self.p_const = ep(tc.tile_pool(name="const", bufs=1))
        self.p_s512 = ep(tc.tile_pool(name="s512", bufs=8))    # residual streams
        self.p_misc = ep(tc.tile_pool(name="misc", bufs=6))
        self.p_bc = ep(tc.tile_pool(name="bcst", bufs=4))
        self.p_row = ep(tc.tile_pool(name="rows", bufs=4))
        self.p_ps = ep(tc.tile_pool(name="ps", bufs=6, space="PSUM"))
        self.p_psr = self.p_ps
        self.p_pst = ep(tc.tile_pool(name="pst", bufs=1, space="PSUM"))
        # scope-dependent pools, assigned by _build_body:
        self.p_s256 = None
        self.p_f32a = None
        self.p_xn = None
        self.p_qkv = None
        self.p_attn = None
        self.p_w = None
        self.p_out = None

        self.ident = self.p_const.tile([P, P], F32, tag="identf")
        make_identity(nc, self.ident[:])
        self.ident16 = self.p_const.tile([P, P], BF16, tag="identb")
        nc.vector.tensor_copy(self.ident16[:], self.ident[:])
        ones32 = self.p_const.tile([P, 1], F32, tag="ones32")
        nc.vector.memset(ones32[:], 1.0)
        self.ones = self.p_const.tile([P, 1], F32R, tag="ones")
        nc.vector.tensor_copy(self.ones[:], ones32[:])
        self.ones16 = self.p_const.tile([P, 1], BF16, tag="ones16")
        nc.vector.memset(self.ones16[:], 1.0)
        self.c_pihalf = self.p_const.tile([P, 1], F32, tag="cpih")
        nc.vector.memset(self.c_pihalf[:], float(np.pi / 2))
        self.c_eps = self.p_const.tile([P, 1], F32, tag="ceps")
        nc.vector.memset(self.c_eps[:], EPS)
        self.mask_top = self.p_const.tile([1, P], F32, tag="mtop")
        nc.vector.memset(self.mask_top[:], 0.0)
        nc.vector.memset(self.mask_top[0:1, 0:64], 1.0)
        self.mask_bot = self.p_const.tile([1, P], F32, tag="mbot")
        nc.vector.memset(self.mask_bot[:], 0.0)
        nc.vector.memset(self.mask_bot[0:1, 64:128], 1.0)

    def load_w(self, dram):
        t = self.p_w.tile([P, NCH * D], BF16, tag="wt")
        self.nc.sync.dma_start(t[:], dram.rearrange("p c d -> p (c d)"))
        return t

    def wslice(self, w, c, m=None, width=P):
        if m is None:
            return w[:, D * c:D * (c + 1)]
        return w[:, D * c + P * m: D * c + P * m + width]

    def ln_stats_B(self, xs, ntok):
        """Single-part wrapper around ln_stats2_B."""
        (rb, nb), _ = self.ln_stats2_B(xs, None, ntok)
        return rb, nb

    def ln_stats2_B(self, xs_r, xs_i, ntok):
        """LN stats over D for one or two layout-B tile sets. The Ln/Exp
        rstd chain runs once for both parts via a stride-32 partition AP."""
        nc = self.nc
        both = xs_i is not None
        nparts = 2 if both else 1
        rows = 33 if both else 1
        parts = [xs_r] + ([xs_i] if both else [])
        ps_sums = []
        for xs in parts:
            ps_s = self.p_ps.tile([1, ntok], F32, tag="ps", space="PSUM",
                                  name="ps_s")
            for c in range(NCH):
                nc.tensor.matmul(ps_s[:], self.ones[:, 0:1], xs[c][:],
                                 start=(c == 0), stop=(c == NCH - 1))
            ps_q = self.p_ps.tile([1, ntok], F32, tag="ps", space="PSUM",
                                  name="ps_q")
            for c in range(NCH):
                sq = self.p_misc.tile([P, ntok], F32R, tag="sq", bufs=5)
                nc.scalar.activation(sq[:], xs[c][:], AF.Square)
                nc.tensor.matmul(ps_q[:], self.ones[:, 0:1], sq[:],
                                 start=(c == 0), stop=(c == NCH - 1))
            ps_sums.append((ps_s, ps_q))

        mean = self.p_row.tile([rows, ntok], F32, tag="row", bufs=4)
        var = self.p_row.tile([rows, ntok], F32, tag="row", bufs=4)
        if both:
            nc.vector.memset(mean[:], 1.0)
            nc.vector.memset(var[:], 1.0)
        for pi, (ps_s, ps_q) in enumerate(ps_sums):
            sl = slice(32 * pi, 32 * pi + 1)
            nc.vector.tensor_scalar(out=mean[sl, :], in0=ps_s[:],
                                    scalar1=1.0 / D, scalar2=None, op0=OP.mult)
            m2 = self.p_row.tile([1, ntok], F32, tag="row", bufs=4)
            nc.vector.tensor_tensor(out=m2[:], in0=mean[sl, :],
                                    in1=mean[sl, :], op=OP.mult)
            nc.vector.scalar_tensor_tensor(out=var[sl, :], in0=ps_q[:],
                                           scalar=1.0 / D, in1=m2[:],
                                           op0=OP.mult, op1=OP.subtract)
        def sv(t):
            return t[:]
        lnv = self.p_row.tile([rows, ntok], F32, tag="row", bufs=4)
        nc.scalar.activation(sv(lnv), sv(var), AF.Ln,
                             bias=self.c_eps[0:rows, :])
        rstd = self.p_row.tile([rows, ntok], F32, tag="row", bufs=4)
        nc.scalar.activation(sv(rstd), sv(lnv), AF.Exp, scale=-0.5)
        nmr = self.p_row.tile([rows, ntok], F32, tag="row", bufs=4)
        nc.vector.scalar_tensor_tensor(out=sv(nmr), in0=sv(mean), scalar=-1.0,
                                       in1=sv(rstd), op0=OP.mult, op1=OP.mult)
        outs = []
        for pi in range(nparts):
            sl = slice(32 * pi, 32 * pi + 1)
            # partition_broadcast reads absolute partition 0 -> copy row 32
            # down to a base-0 tile first.
            if pi == 0:
                rsrc, nsrc = rstd[sl, :], nmr[sl, :]
            else:
                r0 = self.p_row.tile([1, ntok], F32, tag="row", bufs=4)
                nc.vector.tensor_copy(r0[:], rstd[sl, :])
                n0 = self.p_row.tile([1, ntok], F32, tag="row", bufs=4)
                nc.vector.tensor_copy(n0[:], nmr[sl, :])
                rsrc, nsrc = r0[:], n0[:]
            rstd_b = self.p_bc.tile([P, ntok], F32, tag="bcast", bufs=4,
                                    name=f"rstdb{pi}")
            nc.gpsimd.partition_broadcast(rstd_b[:], rsrc)
            nmr_b = self.p_bc.tile([P, ntok], F32, tag="bcast", bufs=4,
                                   name=f"nmrb{pi}")
            nc.gpsimd.partition_broadcast(nmr_b[:], nsrc)
            outs.append((rstd_b, nmr_b))
        return outs[0], (outs[1] if both else None)

    def ln_apply_B(self, xs, rstd_b, nmr_b, ntok, out_dtype=BF16, out_pool=None,
                   out_tag="xn", gb=None):
        nc = self.nc
        out_pool = out_pool or self.p_xn
        outs = []
        for c in range(NCH):
            t1 = self.p_misc.tile([P, ntok], F32, tag="sq", bufs=5)
            nc.vector.tensor_tensor(out=t1[:], in0=xs[c][:], in1=rstd_b[:],
                                    op=OP.mult)
            if gb is None:
                o = out_pool.tile([P, ntok], out_dtype, tag=out_tag)
                nc.gpsimd.tensor_tensor(out=o[:], in0=t1[:], in1=nmr_b[:],
                                        op=OP.add)
            else:
                t2 = self.p_misc.tile([P, ntok], F32, tag="sq", bufs=5)
                nc.gpsimd.tensor_tensor(out=t2[:], in0=t1[:], in1=nmr_b[:],
                                        op=OP.add)
                o = out_pool.tile([P, ntok], out_dtype, tag=out_tag)
                g_t, b_t = gb
                nc.scalar.activation(o[:], t2[:], AF.Identity,
                                     scale=g_t[:, c:c + 1], bias=b_t[:, c:c + 1])
            outs.append(o)
        return outs


def build_nc(debug=False):
    nc = bacc.Bacc("TRN2", target_bir_lowering=False, debug=False)

    din = {}
    def dram_in(name, shape, dtype):
        din[name] = nc.dram_tensor(name, list(shape), dtype,
                                   kind="ExternalInput").ap()

    dram_in("xidx", (TOK, 1), I32)
    dram_in("emb_real", (V, D), F32)
    dram_in("emb_imag", (V, D), F32)
    dram_in("freq", (1, D), F32)
    dram_in("srange", (P, S // P), F32)
    for ln_ in LAYERS:
        for wn in ("q", "k"):
            dram_in(f"{ln_}_{wn}T", (P, NCH, D), BF16)
            dram_in(f"{ln_}_{wn}b", (P, H), F32)
        dram_in(f"{ln_}_vT", (P, NCH, D), BF16)
        dram_in(f"{ln_}_vbrow", (1, D), F32)
        dram_in(f"{ln_}_oT", (P, NCH, D), BF16)
        dram_in(f"{ln_}_ob", (P, NCH), F32)
    for nm in ("wp", "ps", "rd"):
        dram_in(f"{nm}T", (P, NCH, D), BF16)
        dram_in(f"{nm}b", (P, NCH), F32)
    dram_in("memT", (P, NCH, D), BF16)
    dram_in("mem_r", (P, NCH, D), BF16)
    dram_in("mem_i", (P, NCH, D), BF16)
    for nm in ("ln_model_g", "ln_model_b", "ln_hier_g", "ln_hier_b"):
        dram_in(nm, (P, NCH), F32)
    dram_in("cwT", (P, NVT, D), BF16)
    dram_in("cb", (P, NVT), F32)

    out_dram = nc.dram_tensor("out", [V, TOK], BF16, kind="ExternalOutput").ap()
    taps = {}
    if debug:
        for nm in ("dbg_hr", "dbg_hi", "dbg_wordr", "dbg_wordi",
                   "dbg_fusedr", "dbg_fusedi", "dbg_density"):
            taps[nm] = nc.dram_tensor(nm, [D, TOK], F32,
                                      kind="ExternalOutput").ap()
        for nm in ("dbg_xnr", "dbg_rstdb", "dbg_q0", "dbg_k0", "dbg_vt0",
                   "dbg_exp0", "dbg_rb0", "dbg_attnr0", "dbg_av0"):
            taps[nm] = nc.dram_tensor(nm, [P, TOK], F32,
                                      kind="ExternalOutput").ap()

    with tile.TileContext(nc) as tc:
        with ExitStack() as ctx:
            k = K(ctx, tc)
            _build_body(k, din, out_dram, taps)
    nc.compile()
    return nc


def _tap(k, taps, name, xs):
    if name in taps:
        for c in range(NCH):
            k.nc.sync.dma_start(taps[name][P * c:P * (c + 1), :],
                                xs[c][:].bitcast(F32))


def _build_body(k, din, out_dram, taps):
    nc = k.nc

    # ---- stage A: embedding + positional + initial cln (token-major) ----
    idx = k.p_const.tile([P, NCH], I32, tag="idx")
    nc.sync.dma_start(idx[:], din["xidx"][:, 0].rearrange("(t p) -> p t", p=P))

    freq_row = k.p_const.tile([1, D], F32, tag="freqr")
    nc.sync.dma_start(freq_row[:], din["freq"][:])
    freq_b = k.p_const.tile([P, D], F32, tag="freqb")
    nc.gpsimd.partition_broadcast(freq_b[:], freq_row[:])
    srange = k.p_const.tile([P, S // P], F32, tag="srange")
    nc.sync.dma_start(srange[:], din["srange"][:])

    pe = {}
    for j in range(S // P):
        ang = k.p_misc.tile([P, D], F32, tag="sq")
        nc.vector.tensor_scalar(out=ang[:], in0=freq_b[:],
                                scalar1=srange[:, j:j + 1], scalar2=None,
                                op0=OP.mult)
        pr = k.p_misc.tile([P, D], F32, tag="pe")
        nc.scalar.activation(pr[:], ang[:], AF.Sin, bias=k.c_pihalf[:])
        pi = k.p_misc.tile([P, D], F32, tag="pe")
        nc.scalar.activation(pi[:], ang[:], AF.Sin)
        pe[j] = {"r": pr, "i": pi}

    g_model = k.p_const.tile([P, NCH], F32, tag="gmod")
    nc.sync.dma_start(g_model[:], din["ln_model_g"][:])
    b_model = k.p_const.tile([P, NCH], F32, tag="bmod")
    nc.sync.dma_start(b_model[:], din["ln_model_b"][:])
    g_hier = k.p_const.tile([P, NCH], F32, tag="ghier")
    nc.sync.dma_start(g_hier[:], din["ln_hier_g"][:])
    b_hier = k.p_const.tile([P, NCH], F32, tag="bhier")
    nc.sync.dma_start(b_hier[:], din["ln_hier_b"][:])

    h_B = {"r": [], "i": []}
    for part in ("r", "i"):
        for c in range(NCH):
            h_B[part].append(k.p_s512.tile([P, TOK], F32R, tag=f"st512{part}", name=f"hB{part}{c}"))

    for t in range(TOK // P):
        for part, tbl in (("r", "emb_real"), ("i", "emb_imag")):
            emb = k.p_misc.tile([P, D], F32, tag="sq")
            nc.gpsimd.indirect_dma_start(
                out=emb[:], out_offset=None, in_=din[tbl][:],
                in_offset=bass.IndirectOffsetOnAxis(ap=idx[:, t:t + 1], axis=0))
            hh = k.p_misc.tile([P, D], F32, tag="htm")
            nc.vector.tensor_tensor(out=hh[:], in0=emb[:],
                                    in1=pe[t % 2][part][:], op=OP.add)
            ssum = k.p_row.tile([P, 1], F32, tag="cst")
            nc.vector.reduce_sum(ssum[:], hh[:], AX.X)
            sqscr = k.p_misc.tile([P, D], F32, tag="sq")
            ssq = k.p_row.tile([P, 1], F32, tag="cst")
            nc.scalar.activation(sqscr[:], hh[:], AF.Square, accum_out=ssq[:])
            mean = k.p_row.tile([P, 1], F32, tag="cst")
            nc.vector.tensor_scalar(out=mean[:], in0=ssum[:], scalar1=1.0 / D,
                                    scalar2=None, op0=OP.mult)
            m2 = k.p_row.tile([P, 1], F32, tag="cst")
            nc.vector.tensor_tensor(out=m2[:], in0=mean[:], in1=mean[:],
                                    op=OP.mult)
            var = k.p_row.tile([P, 1], F32, tag="cst")
            nc.vector.scalar_tensor_tensor(out=var[:], in0=ssq[:],
                                           scalar=1.0 / D, in1=m2[:],
                                           op0=OP.mult, op1=OP.subtract)
            lnv = k.p_row.tile([P, 1], F32, tag="cst")
            nc.scalar.activation(lnv[:], var[:], AF.Ln, bias=k.c_eps[:])
            rstd = k.p_row.tile([P, 1], F32, tag="cst")
            nc.scalar.activation(rstd[:], lnv[:], AF.Exp, scale=-0.5)
            nmr = k.p_row.tile([P, 1], F32, tag="cst")
            nc.vector.scalar_tensor_tensor(out=nmr[:], in0=mean[:], scalar=-1.0,
                                           in1=rstd[:], op0=OP.mult, op1=OP.mult)
            xn = k.p_misc.tile([P, D], F32, tag="htm")
            nc.scalar.activation(xn[:], hh[:], AF.Identity, scale=rstd[:],
                                 bias=nmr[:])
            for c in range(NCH):
                pst = k.p_pst.tile([P, P], F32, tag="pst", space="PSUM")
                nc.tensor.transpose(pst[:], xn[:, P * c:P * (c + 1)], k.ident[:])
                nc.scalar.activation(h_B[part][c][:, P * t:P * (t + 1)], pst[:],
                                     AF.Identity, scale=g_model[:, c:c + 1],
                                     bias=b_model[:, c:c + 1])

    _tap(k, taps, "dbg_hr", h_B["r"])
    _tap(k, taps, "dbg_hi", h_B["i"])

    # ---- attention layer ----
    def attn_layer(prefix, xs_r, xs_i, ntok, seqlen, spool):
        n_kch = seqlen // P              # ki chunks per sequence (2/1/1)
        nsb = seqlen // P

        (rstd_br, nmr_br), ri = k.ln_stats2_B(xs_r, xs_i, ntok)
        xn_r = k.ln_apply_B(xs_r, rstd_br, nmr_br, ntok)
        xn_i = k.ln_apply_B(xs_i, ri[0], ri[1], ntok)

        wq = k.load_w(din[f"{prefix}_qT"])
        wk = k.load_w(din[f"{prefix}_kT"])
        wv = k.load_w(din[f"{prefix}_vT"])
        wo = k.load_w(din[f"{prefix}_oT"])
        qb = k.p_w.tile([P, H], F32, tag="hb", bufs=6)
        nc.sync.dma_start(qb[:], din[f"{prefix}_qb"][:])
        kb = k.p_w.tile([P, H], F32, tag="hb", bufs=6)
        nc.sync.dma_start(kb[:], din[f"{prefix}_kb"][:])
        ob = k.p_w.tile([P, NCH], F32, tag="hb", bufs=6)
        nc.sync.dma_start(ob[:], din[f"{prefix}_ob"][:])
        vb_row = k.p_row.tile([1, D], F32, tag="vbrow", bufs=2)
        nc.sync.dma_start(vb_row[:], din[f"{prefix}_vbrow"][:])
        vb_b = k.p_bc.tile([P, D], F32, tag="vbb", bufs=2)
        nc.gpsimd.partition_broadcast(vb_b[:], vb_row[:])

        def qk_cat(w, bias):
            tiles = []
            for h in range(H):
                ps = k.p_ps.tile([P, ntok], F32, tag="ps", space="PSUM")
                for c in range(NCH):
                    nc.tensor.matmul(
                        ps[0:64, :], k.wslice(w, c)[:, 64 * h:64 * h + 64],
                        xn_r[c][:], start=(c == 0), stop=(c == NCH - 1),
                        tile_position=(0, 0))
                for c in range(NCH):
                    nc.tensor.matmul(
                        ps[64:128, :], k.wslice(w, c)[:, 64 * h:64 * h + 64],
                        xn_i[c][:], start=(c == 0), stop=(c == NCH - 1),
                        tile_position=(0, 64))
                t = k.p_qkv.tile([P, ntok], BF16, tag="qkcat", bufs=18)
                nc.vector.tensor_scalar(out=t[:], in0=ps[:],
                                        scalar1=bias[:, h:h + 1], scalar2=None,
                                        op0=OP.add)
                tiles.append(t)
            return tiles

        q_cat = qk_cat(wq, qb)
        k_cat = qk_cat(wk, kb)

        vT = {"r": [], "i": []}
        for part, xn in (("r", xn_r), ("i", xn_i)):
            for m in range(ntok // P):
                ps = k.p_ps.tile([P, D], F32, tag="ps", space="PSUM")
                for c in range(NCH):
                    nc.tensor.matmul(ps[:], xn[c][:, P * m:P * (m + 1)],
                                     k.wslice(wv, c), start=(c == 0),
                                     stop=(c == NCH - 1))
                t = k.p_qkv.tile([P, D], BF16, tag="vt", bufs=8)
                nc.vector.tensor_tensor(out=t[:], in0=ps[:], in1=vb_b[:],
                                        op=OP.add)
                vT[part].append(t)

        attn_s = {"r": [], "i": []}
        for c in range(NCH):
            attn_s["r"].append(k.p_attn.tile([P, ntok], BF16, tag="attnr",
                                             bufs=5, name=f"attnr{c}"))
            attn_s["i"].append(k.p_attn.tile([P, ntok], BF16, tag="attni",
                                             bufs=5, name=f"attni{c}"))

        for b in range(BPC):
            sl = slice(seqlen * b, seqlen * (b + 1))
            for h in range(H):
                sums = k.p_psr.tile([1, seqlen], F32, tag="psrow", space="PSUM")
                expT = []
                for j in range(n_kch):
                    pss = k.p_ps.tile([P, seqlen], F32, tag="ps", space="PSUM")
                    nc.tensor.matmul(
                        pss[:],
                        k_cat[h][:, seqlen * b + P * j:seqlen * b + P * (j + 1)],
                        q_cat[h][:, sl], start=True, stop=True)
                    e = k.p_attn.tile([P, seqlen], BF16, tag="expT", bufs=6)
                    nc.scalar.activation(e[:], pss[:], AF.Exp, scale=1.0 / 8.0)
                    expT.append(e)
                    nc.tensor.matmul(sums[:], k.ones16[:, 0:1], e[:],
                                     start=(j == 0), stop=(j == n_kch - 1))
                rr = k.p_misc.tile([1, seqlen], F32, tag="rowpack", bufs=4)
                nc.vector.reciprocal_approx_fast(rr[:], sums[:])
                rb = k.p_attn.tile([P, seqlen], F32, tag="rb", bufs=2)
                nc.gpsimd.partition_broadcast(rb[:], rr[:])

                ps = k.p_ps.tile([P, seqlen], F32, tag="ps", space="PSUM")
                for part, cofs in (("r", 0), ("i", 64)):
                    for j in range(n_kch):
                        nc.tensor.matmul(
                            ps[cofs:cofs + 64, :],
                            vT[part][b * nsb + j][:, 64 * h:64 * h + 64],
                            expT[j][:], start=(j == 0), stop=(j == n_kch - 1),
                            tile_position=(0, cofs))
                cp, half = h // 2, h % 2
                nc.vector.tensor_tensor(
                    out=attn_s["r"][cp][64 * half:64 * half + 64, sl],
                    in0=ps[0:64, :], in1=rb[0:64, :], op=OP.mult)
                nc.vector.tensor_tensor(
                    out=attn_s["i"][cp][64 * half:64 * half + 64, sl],
                    in0=ps[64:128, :], in1=rb[64:128, :], op=OP.mult)

        new_r, new_i = [], []
        for part, attn_t, xs, outl in (("r", attn_s["r"], xs_r, new_r),
                                       ("i", attn_s["i"], xs_i, new_i)):
            for m in range(NCH):
                ps = k.p_ps.tile([P, ntok], F32, tag="ps", space="PSUM")
                for c in range(NCH):
                    nc.tensor.matmul(ps[:], k.wslice(wo, c, m), attn_t[c][:],
                                     start=(c == 0), stop=(c == NCH - 1))
                o = spool.tile([P, ntok], F32R, tag=f"st{ntok}{part}", bufs=8,
                               name=f"res{part}{m}")
                nc.vector.scalar_tensor_tensor(out=o[:], in0=ps[:],
                                               scalar=ob[:, m:m + 1],
                                               in1=xs[m][:], op0=OP.add,
                                               op1=OP.add)
                outl.append(o)
        return new_r, new_i

    def pool2(xs, ntok, dst_pool, part):
        outs = []
        for c in range(NCH):
            o = dst_pool.tile([P, ntok // 2], F32, tag=f"st{ntok // 2}{part}")
            nc.vector.tensor_tensor(out=o[:], in0=xs[c][:, 0:ntok:2],
                                    in1=xs[c][:, 1:ntok:2], op=OP.add)
            nc.vector.tensor_scalar(out=o[:], in0=o[:], scalar1=0.5,
                                    scalar2=None, op0=OP.mult)
            outs.append(o)
        return outs

    def proj_up(prefix, src_r, src_i, base_r, base_i, ntok_half, dst_pool):
        """base + clin_rb(up2(src), proj): bias on real part only."""
        w = k.load_w(din[f"{prefix}T"])
        bsl = k.p_w.tile([P, NCH], F32, tag="hb")
        nc.sync.dma_start(bsl[:], din[f"{prefix}b"][:])
        ntok = ntok_half * 2
        outs = {"r": [], "i": []}
        for part, src, base in (("r", src_r, base_r), ("i", src_i, base_i)):
            src16 = []
            for c in range(NCH):
                t = k.p_xn.tile([P, ntok_half], BF16, tag="xn")
                nc.vector.tensor_copy(t[:], src[c][:])
                src16.append(t)
            for m in range(NCH):
                ps = k.p_ps.tile([P, ntok], F32, tag="ps", space="PSUM")
                for c in range(NCH):
                    rep = src16[c][:].rearrange("p (n o) -> p n o", o=1) \
                        .broadcast_to([P, ntok_half, 2])
                    nc.tensor.matmul(ps[:], k.wslice(w, c, m), rep,
                                     start=(c == 0), stop=(c == NCH - 1))
                o = dst_pool.tile([P, ntok], F32, tag=f"st{ntok}{part}")
                if part == "r":
                    nc.vector.scalar_tensor_tensor(out=o[:], in0=ps[:],
                                                   scalar=bsl[:, m:m + 1],
                                                   in1=base[m][:], op0=OP.add,
                                                   op1=OP.add)
                else:
                    nc.vector.tensor_tensor(out=o[:], in0=ps[:], in1=base[m][:],
                                            op=OP.add)
                outs[part].append(o)
        return outs["r"], outs["i"]

    phr_r = pool2(word_r, TOK, k.p_s256, "r")
    phr_i = pool2(word_i, TOK, k.p_s256, "i")
    for i in range(DEPTH):
        phr_r, phr_i = attn_layer(f"p{i}", phr_r, phr_i, TOK // 2, S // 2,
                                  k.p_s256)

    comb_r, comb_i = proj_up("wp", phr_r, phr_i, word_r, word_i, TOK // 2,
                             k.p_s512)

    sent_r = pool2(comb_r, TOK, k.p_s256, "r")
    sent_i = pool2(comb_i, TOK, k.p_s256, "i")
    for i in range(max(1, DEPTH // 2)):
        sent_r, sent_i = attn_layer(f"s{i}", sent_r, sent_i, TOK // 2, S // 2,
                                    k.p_s256)

    pre_r, pre_i = proj_up("ps", sent_r, sent_i, comb_r, comb_i, TOK // 2,
                           k.p_s512)

    # ---- fused cln (full apply with ln_hier g/b) ----
    rstd_b, nmr_b = k.ln_stats_B(pre_r, TOK)
    fused_r = k.ln_apply_B(pre_r, rstd_b, nmr_b, TOK, out_dtype=F32,
                           out_pool=k.p_f32a, out_tag="fusedr",
                           gb=(g_hier, b_hier))
    rstd_b, nmr_b = k.ln_stats_B(pre_i, TOK)
    fused_i = k.ln_apply_B(pre_i, rstd_b, nmr_b, TOK, out_dtype=F32,
                           out_pool=k.p_f32a, out_tag="fusedi",
                           gb=(g_hier, b_hier))
    _tap(k, taps, "dbg_fusedr", fused_r)
    _tap(k, taps, "dbg_fusedi", fused_i)

    # ---- memory read ----
    fused_r16 = []
    for c in range(NCH):
        t = k.p_xn.tile([P, TOK], BF16, tag="xn")
        nc.vector.tensor_copy(t[:], fused_r[c][:])
        fused_r16.append(t)

    w_memT = k.load_w(din["memT"])
    w_memr = k.load_w(din["mem_r"])
    w_memi = k.load_w(din["mem_i"])

    nsq = []
    for c in range(NCH):
        s1 = k.p_misc.tile([P, TOK], F32, tag="sq")
        nc.scalar.activation(s1[:], fused_r[c][:], AF.Square)
        s2 = k.p_misc.tile([P, TOK], F32, tag="sq")
        nc.scalar.activation(s2[:], fused_i[c][:], AF.Square)
        t = k.p_misc.tile([P, TOK], F32, tag="nsq")
        nc.vector.tensor_tensor(out=t[:], in0=s1[:], in1=s2[:], op=OP.add)
        nsq.append(t)

    wT_mem = []
    for cm in range(NCH):
        wT_mem.append(k.p_qkv.tile([P, TOK], BF16, tag="wTm", name=f"wTm{cm}"))

    for m in range(TOK // P):
        ps_sc = k.p_ps.tile([P, M_MEM], F32, tag="ps", space="PSUM")
        for c in range(NCH):
            nc.tensor.matmul(ps_sc[:], fused_r16[c][:, P * m:P * (m + 1)],
                             k.wslice(w_memT, c), start=(c == 0),
                             stop=(c == NCH - 1))
        nT = k.p_misc.tile([P, D], F32, tag="nT")
        for c in range(NCH):
            pst = k.p_pst.tile([P, P], F32, tag="pst", space="PSUM")
            nc.tensor.transpose(pst[:], nsq[c][:, P * m:P * (m + 1)], k.ident[:])
            nc.scalar.activation(nT[:, P * c:P * (c + 1)], pst[:], AF.Ln)
        nrm = k.p_misc.tile([P, D], F32, tag="nT")
        nc.scalar.activation(nrm[:], nT[:], AF.Exp, scale=0.5)
        nc.vector.tensor_scalar(out=nrm[:], in0=nrm[:], scalar1=1e-8,
                                scalar2=None, op0=OP.add)
        rn = k.p_misc.tile([P, D], F32, tag="nT")
        nc.vector.reciprocal_approx_fast(rn[:], nrm[:])
        z = k.p_misc.tile([P, M_MEM], F32, tag="nT")
        nc.vector.tensor_tensor(out=z[:], in0=ps_sc[:], in1=rn[:], op=OP.mult)
        negmax = k.p_row.tile([P, 1], F32, tag="cst")
        nc.vector.reduce_max(negmax[:], z[:], AX.X, negate=True)
        ez = k.p_misc.tile([P, M_MEM], BF16, tag="ez")
        ssum = k.p_row.tile([P, 1], F32, tag="cst")
        nc.scalar.activation(ez[:], z[:], AF.Exp, bias=negmax[:],
                             accum_out=ssum[:])
        rs = k.p_row.tile([P, 1], F32, tag="cst")
        nc.vector.reciprocal_approx_fast(rs[:], ssum[:])
        wgt = k.p_misc.tile([P, M_MEM], BF16, tag="ez")
        nc.vector.tensor_scalar(out=wgt[:], in0=ez[:], scalar1=rs[:],
                                scalar2=None, op0=OP.mult)
        for cm in range(NCH):
            pst16 = k.p_pst.tile([P, P], BF16, tag="pst", space="PSUM")
            nc.tensor.transpose(pst16[:], wgt[:, P * cm:P * (cm + 1)],
                                k.ident16[:])
            nc.vector.tensor_copy(wT_mem[cm][:, P * m:P * (m + 1)], pst16[:])

    rd_w = k.load_w(din["rdT"])
    rd_b = k.p_w.tile([P, NCH], F32, tag="hb")
    nc.sync.dma_start(rd_b[:], din["rdb"][:])

    h2 = {"r": [], "i": []}
    for part, wmem, fus in (("r", w_memr, fused_r), ("i", w_memi, fused_i)):
        cont16 = []
        for cd in range(NCH):
            ps = k.p_ps.tile([P, TOK], F32, tag="ps", space="PSUM")
            for cm in range(NCH):
                nc.tensor.matmul(ps[:], k.wslice(wmem, cm, cd), wT_mem[cm][:],
                                 start=(cm == 0), stop=(cm == NCH - 1))
            t = k.p_xn.tile([P, TOK], BF16, tag="xn")
            nc.vector.tensor_copy(t[:], ps[:])
            cont16.append(t)
        for m in range(NCH):
            ps = k.p_ps.tile([P, TOK], F32, tag="ps", space="PSUM")
            for c in range(NCH):
                nc.tensor.matmul(ps[:], k.wslice(rd_w, c, m), cont16[c][:],
                                 start=(c == 0), stop=(c == NCH - 1))
            o = k.p_f32a.tile([P, TOK], F32, tag=f"h2{part}")
            nc.vector.scalar_tensor_tensor(out=o[:], in0=ps[:],
                                           scalar=rd_b[:, m:m + 1],
                                           in1=fus[m][:], op0=OP.add, op1=OP.add)
            h2[part].append(o)

    # ---- final cln + density ----
    rstd_b, nmr_b = k.ln_stats_B(h2["r"], TOK)
    hn_r = k.ln_apply_B(h2["r"], rstd_b, nmr_b, TOK, out_dtype=F32,
                        out_pool=k.p_f32a, out_tag="hn")
    rstd_b, nmr_b = k.ln_stats_B(h2["i"], TOK)
    hn_i = k.ln_apply_B(h2["i"], rstd_b, nmr_b, TOK, out_dtype=F32,
                        out_pool=k.p_f32a, out_tag="hn")
    density16 = []
    for c in range(NCH):
        d1 = k.p_misc.tile([P, TOK], F32, tag="sq")
        nc.scalar.activation(d1[:], hn_r[c][:], AF.Square,
                             scale=g_model[:, c:c + 1], bias=b_model[:, c:c + 1])
        d2 = k.p_misc.tile([P, TOK], F32, tag="sq")
        nc.scalar.activation(d2[:], hn_i[c][:], AF.Square,
                             scale=g_model[:, c:c + 1], bias=b_model[:, c:c + 1])
        dt_ = k.p_misc.tile([P, TOK], BF16, tag="dens")
        nc.vector.tensor_tensor(out=dt_[:], in0=d1[:], in1=d2[:], op=OP.add)
        density16.append(dt_)
    if "dbg_density" in taps:
        for c in range(NCH):
            f32t = k.p_misc.tile([P, TOK], F32, tag="sq")
            nc.vector.tensor_copy(f32t[:], density16[c][:])
            nc.sync.dma_start(taps["dbg_density"][P * c:P * (c + 1), :], f32t[:])

    # ---- collapse ----
    cb_t = k.p_const.tile([P, NVT], F32, tag="cb")
    nc.sync.dma_start(cb_t[:], din["cb"][:])
    for t in range(NVT):
        cw = k.p_w.tile([P, D], BF16, tag="cw")
        nc.sync.dma_start(cw[:], din["cwT"][:, t, :])
        ps = k.p_ps.tile([P, TOK], F32, tag="ps", space="PSUM")
        for c in range(NCH):
            nc.tensor.matmul(ps[:], cw[:, P * c:P * (c + 1)], density16[c][:],
                             start=(c == 0), stop=(c == NCH - 1))
        o = k.p_out.tile([P, TOK], BF16, tag="out")
        nc.scalar.activation(o[:], ps[:], AF.Relu, bias=cb_t[:, t:t + 1])
        nc.sync.dma_start(out_dram[P * t:P * (t + 1), :], o[:])


# ----------------------------------------------------------------------------
# entry point
# ----------------------------------------------------------------------------

_RUN_KW = {}


def kernel(x=None, params=None, **kw):
    if x is None:
        x = kw.pop("x")
    if params is None:
        params = kw.pop("params")
    in_maps = _prep(x, params)
    nc = build_nc(debug=DEBUG)
    res = run_bass_kernel_spmd(nc, in_maps, core_ids=list(range(NCORES)),
                               **_RUN_KW)
    outs = []
    for c in range(NCORES):
        o = np.asarray(res.results[c]["out"])          # [V, TOK] bf16
        o = o.reshape(V, BPC, S).transpose(1, 2, 0).astype(np.float32)
        outs.append(o)
    full = np.concatenate(outs, axis=0)                # [B, S, V]
    kernel.last_results = res
    return full


# revision 28
# speedup vs baseline: 1.0023x; 1.0023x over previous
"""Trainium2 Bass kernel for nn_AdvancedSFIN (hierarchical complex transformer).

Self-contained: builds a single-core Bass/Tile program, runs it SPMD on 8
NeuronCores (data-parallel over batch: 2 sequences per core), reassembles the
full [16, 256, 32000] float32 output on the host.

Design notes:
- Activations live in "layout B": [D(partitions, 4 chunks of 128), tok(free)].
  Linear layers contract over D natively (lhsT = host-pre-transposed weights).
- All matmuls run in bf16 (validated ~3.4e-3 scale-relative error end to end);
  LayerNorm statistics matmuls use float32r (full-rate fp32 path).
- LayerNorm over D (partition axis) uses ones-vector matmuls for sum/sum-sq,
  gpsimd partition_broadcast for the per-token stats, and exp(-0.5*ln(var+eps))
  on the Scalar engine for rsqrt (keeps the natural_log_exp ACT table resident;
  sqrt never appears on ACT).
- Attention per (seq, head): scoresT = k_cat.T @ q_cat with [real|imag]
  concatenated on the contraction axis; softmax over the partition (ki) axis
  via ones-matmul sums (scores are bounded ~|1.7| so no max subtraction);
  AV uses token-major V (produced directly by using the normalized input as
  the stationary matmul operand), real/imag packed into one PSUM tile by
  column tiling; the 1/sum normalization rides on PSUM eviction.
- ln_hier gamma/beta are folded into the QKV weights on the host (the cln
  ahead of each attention feeds only those linears).
- Memory read + collapse produce logits v-major [V, tok] so the collapse bias
  + relu(+1e-10 folded into the bias) ride per-partition on ScalarE; the host
  transposes back and upcasts bf16 -> f32.
"""

import numpy as np
import ml_dtypes
from contextlib import ExitStack

import concourse.bass as bass
import concourse.bacc as bacc
import concourse.tile as tile
from concourse import mybir
from concourse.bass_utils import run_bass_kernel_spmd
from concourse.masks import make_identity

P = 128
V, D, H, M_MEM = 32000, 512, 8, 512
B, S = 16, 256
DEPTH = 2
NCORES = 8
BPC = B // NCORES          # sequences per core
TOK = BPC * S              # tokens per core (512)
HD = D // H                # head dim (64)
NCH = D // P               # d-chunks (4)
NVT = V // P               # collapse v-tiles (250)
EPS = 1e-5

F32 = mybir.dt.float32
F32R = mybir.dt.float32r
BF16 = mybir.dt.bfloat16
I32 = mybir.dt.int32
AF = mybir.ActivationFunctionType
OP = mybir.AluOpType
AX = mybir.AxisListType

BF = ml_dtypes.bfloat16

DEBUG = False  # when True, adds DRAM taps for intermediates (dev only)

LAYERS = [f"w{i}" for i in range(DEPTH)] + \
         [f"p{i}" for i in range(DEPTH)] + \
         [f"s{i}" for i in range(max(1, DEPTH // 2))]


# ----------------------------------------------------------------------------
# host-side prep
# ----------------------------------------------------------------------------

def _wt_tiled(w_t: np.ndarray) -> np.ndarray:
    """[din, dout] -> [128, din//128, dout] bf16 (k-chunk c at [:, c, :])."""
    din, dout = w_t.shape
    return np.ascontiguousarray(
        w_t.reshape(din // P, P, dout).transpose(1, 0, 2)).astype(BF)


def _bias_pp(b: np.ndarray) -> np.ndarray:
    """[512] -> [128, 4] f32 per-partition layout (chunk c in column c)."""
    return np.ascontiguousarray(b.reshape(-1, P).T).astype(np.float32)


def _prep(x: np.ndarray, params: dict):
    gh = np.asarray(params["ln_hier_g"], np.float32)
    bh = np.asarray(params["ln_hier_b"], np.float32)

    shared = {}
    shared["emb_real"] = np.ascontiguousarray(np.asarray(params["emb_real"], np.float32))
    shared["emb_imag"] = np.ascontiguousarray(np.asarray(params["emb_imag"], np.float32))
    shared["freq"] = np.ascontiguousarray(
        np.asarray(params["freq"], np.float32).reshape(1, D))
    sr = np.arange(S, dtype=np.float32).reshape(S // P, P).T
    shared["srange"] = np.ascontiguousarray(sr)

    def attn_prep(prefix, p):
        for wname in ("q", "k"):
            w = np.asarray(p[wname]["W"], np.float32)
            b = np.asarray(p[wname]["b"], np.float32)
            w_eff = w * gh[None, :]
            b_eff = b + w @ bh
            shared[f"{prefix}_{wname}T"] = _wt_tiled(w_eff.T)
            bb = np.empty((P, H), np.float32)
            for h in range(H):
                bb[0:64, h] = b_eff[64 * h:64 * h + 64]
                bb[64:128, h] = b_eff[64 * h:64 * h + 64]
            shared[f"{prefix}_{wname}b"] = np.ascontiguousarray(bb)
        w = np.asarray(p["v"]["W"], np.float32)
        b = np.asarray(p["v"]["b"], np.float32)
        shared[f"{prefix}_vT"] = _wt_tiled((w * gh[None, :]).T)
        shared[f"{prefix}_vbrow"] = np.ascontiguousarray(
            (b + w @ bh).reshape(1, D)).astype(np.float32)
        w = np.asarray(p["o"]["W"], np.float32)
        b = np.asarray(p["o"]["b"], np.float32)
        shared[f"{prefix}_oT"] = _wt_tiled(w.T)
        shared[f"{prefix}_ob"] = _bias_pp(b)

    for prefix, p in zip(LAYERS, list(params["word"]) + list(params["phrase"])
                         + list(params["sentence"])):
        attn_prep(prefix, p)

    for nm, key in (("wp", "proj_wp"), ("ps", "proj_ps"), ("rd", "read")):
        w = np.asarray(params[key]["W"], np.float32)
        b = np.asarray(params[key]["b"], np.float32)
        shared[f"{nm}T"] = _wt_tiled(w.T)
        shared[f"{nm}b"] = _bias_pp(b)

    mem_r = np.asarray(params["mem_real"], np.float32)
    mem_i = np.asarray(params["mem_imag"], np.float32)
    shared["memT"] = _wt_tiled(mem_r.T)
    shared["mem_r"] = _wt_tiled(mem_r)
    shared["mem_i"] = _wt_tiled(mem_i)

    shared["ln_model_g"] = _bias_pp(np.asarray(params["ln_model_g"], np.float32))
    shared["ln_model_b"] = _bias_pp(np.asarray(params["ln_model_b"], np.float32))
    shared["ln_hier_g"] = _bias_pp(gh)
    shared["ln_hier_b"] = _bias_pp(bh)

    cw = np.asarray(params["collapse_W"], np.float32)   # [V, D]
    cb = np.asarray(params["collapse_b"], np.float32)
    A = cw.reshape(NVT, P, NCH, P).transpose(3, 0, 2, 1)   # [p, t, c, v']
    shared["cwT"] = np.ascontiguousarray(A.reshape(P, NVT, D)).astype(BF)
    shared["cb"] = np.ascontiguousarray(
        (cb + 1e-10).reshape(NVT, P).T).astype(np.float32)

    x = np.asarray(x).astype(np.int32)
    per_core = []
    for c in range(NCORES):
        m = dict(shared)
        m["xidx"] = np.ascontiguousarray(x[BPC * c:BPC * (c + 1)].reshape(TOK, 1))
        per_core.append(m)
    return per_core


# ----------------------------------------------------------------------------
# device program
# ----------------------------------------------------------------------------

class K:
    def __init__(self, ctx, tc):
        self.ctx = ctx
        self.tc = tc
        self.nc = tc.nc
        nc = self.nc
        ep = ctx.enter_context

        self.p_const = ep(tc.tile_pool(name="const", bufs=1))
        self.p_s512 = ep(tc.tile_pool(name="s512", bufs=8))    # residual streams
        self.p_misc = ep(tc.tile_pool(name="misc", bufs=6))
        self.p_bc = ep(tc.tile_pool(name="bcst", bufs=4))
        self.p_row = ep(tc.tile_pool(name="rows", bufs=4))
        self.p_ps = ep(tc.tile_pool(name="ps", bufs=6, space="PSUM"))
        self.p_psr = self.p_ps
        self.p_pst = ep(tc.tile_pool(name="pst", bufs=1, space="PSUM"))
        # scope-dependent pools, assigned by _build_body:
        self.p_s256 = None
        self.p_f32a = None
        self.p_xn = None
        self.p_qkv = None
        self.p_attn = None
        self.p_w = None
        self.p_out = None

        self.ident = self.p_const.tile([P, P], F32, tag="identf")
        make_identity(nc, self.ident[:])
        self.ident16 = self.p_const.tile([P, P], BF16, tag="identb")
        nc.vector.tensor_copy(self.ident16[:], self.ident[:])
        ones32 = self.p_const.tile([P, 1], F32, tag="ones32")
        nc.vector.memset(ones32[:], 1.0)
        self.ones = self.p_const.tile([P, 1], F32R, tag="ones")
        nc.vector.tensor_copy(self.ones[:], ones32[:])
        self.ones16 = self.p_const.tile([P, 1], BF16, tag="ones16")
        nc.vector.memset(self.ones16[:], 1.0)
        self.c_pihalf = self.p_const.tile([P, 1], F32, tag="cpih")
        nc.vector.memset(self.c_pihalf[:], float(np.pi / 2))
        self.c_eps = self.p_const.tile([P, 1], F32, tag="ceps")
        nc.vector.memset(self.c_eps[:], EPS)
        self.mask_top = self.p_const.tile([1, P], F32, tag="mtop")
        nc.vector.memset(self.mask_top[:], 0.0)
        nc.vector.memset(self.mask_top[0:1, 0:64], 1.0)
        self.mask_bot = self.p_const.tile([1, P], F32, tag="mbot")
        nc.vector.memset(self.mask_bot[:], 0.0)
        nc.vector.memset(self.mask_bot[0:1, 64:128], 1.0)

    def load_w(self, dram):
        t = self.p_w.tile([P, NCH * D], BF16, tag="wt")
        self.nc.sync.dma_start(t[:], dram.rearrange("p c d -> p (c d)"))
        return t

    def wslice(self, w, c, m=None, width=P):
        if m is None:
            return w[:, D * c:D * (c + 1)]
        return w[:, D * c + P * m: D * c + P * m + width]

    def ln_stats_B(self, xs, ntok):
        """Single-part wrapper around ln_stats2_B."""
        (rb, nb), _ = self.ln_stats2_B(xs, None, ntok)
        return rb, nb

    def ln_stats2_B(self, xs_r, xs_i, ntok):
        """LN stats over D for one or two layout-B tile sets. The Ln/Exp
        rstd chain runs once for both parts via a stride-32 partition AP."""
        nc = self.nc
        both = xs_i is not None
        nparts = 2 if both else 1
        rows = 33 if both else 1
        parts = [xs_r] + ([xs_i] if both else [])
        ps_sums = []
        for xs in parts:
            ps_s = self.p_ps.tile([1, ntok], F32, tag="ps", space="PSUM",
                                  name="ps_s")
            for c in range(NCH):
                nc.tensor.matmul(ps_s[:], self.ones[:, 0:1], xs[c][:],
                                 start=(c == 0), stop=(c == NCH - 1))
            ps_q = self.p_ps.tile([1, ntok], F32, tag="ps", space="PSUM",
                                  name="ps_q")
            for c in range(NCH):
                sq = self.p_misc.tile([P, ntok], F32R, tag="sq", bufs=5)
                nc.scalar.activation(sq[:], xs[c][:], AF.Square)
                nc.tensor.matmul(ps_q[:], self.ones[:, 0:1], sq[:],
                                 start=(c == 0), stop=(c == NCH - 1))
            ps_sums.append((ps_s, ps_q))

        mean = self.p_row.tile([rows, ntok], F32, tag="row", bufs=4)
        var = self.p_row.tile([rows, ntok], F32, tag="row", bufs=4)
        if both:
            nc.vector.memset(mean[:], 1.0)
            nc.vector.memset(var[:], 1.0)
        for pi, (ps_s, ps_q) in enumerate(ps_sums):
            sl = slice(32 * pi, 32 * pi + 1)
            nc.vector.tensor_scalar(out=mean[sl, :], in0=ps_s[:],
                                    scalar1=1.0 / D, scalar2=None, op0=OP.mult)
            m2 = self.p_row.tile([1, ntok], F32, tag="row", bufs=4)
            nc.vector.tensor_tensor(out=m2[:], in0=mean[sl, :],
                                    in1=mean[sl, :], op=OP.mult)
            nc.vector.scalar_tensor_tensor(out=var[sl, :], in0=ps_q[:],
                                           scalar=1.0 / D, in1=m2[:],
                                           op0=OP.mult, op1=OP.subtract)
        def sv(t):
            return t[:]
        lnv = self.p_row.tile([rows, ntok], F32, tag="row", bufs=4)
        nc.scalar.activation(sv(lnv), sv(var), AF.Ln,
                             bias=self.c_eps[0:rows, :])
        rstd = self.p_row.tile([rows, ntok], F32, tag="row", bufs=4)
        nc.scalar.activation(sv(rstd), sv(lnv), AF.Exp, scale=-0.5)
        nmr = self.p_row.tile([rows, ntok], F32, tag="row", bufs=4)
        nc.vector.scalar_tensor_tensor(out=sv(nmr), in0=sv(mean), scalar=-1.0,
                                       in1=sv(rstd), op0=OP.mult, op1=OP.mult)
        outs = []
        for pi in range(nparts):
            sl = slice(32 * pi, 32 * pi + 1)
            # partition_broadcast reads absolute partition 0 -> copy row 32
            # down to a base-0 tile first.
            if pi == 0:
                rsrc, nsrc = rstd[sl, :], nmr[sl, :]
            else:
                r0 = self.p_row.tile([1, ntok], F32, tag="row", bufs=4)
                nc.vector.tensor_copy(r0[:], rstd[sl, :])
                n0 = self.p_row.tile([1, ntok], F32, tag="row", bufs=4)
                nc.vector.tensor_copy(n0[:], nmr[sl, :])
                rsrc, nsrc = r0[:], n0[:]
            rstd_b = self.p_bc.tile([P, ntok], F32, tag="bcast", bufs=4,
                                    name=f"rstdb{pi}")
            nc.gpsimd.partition_broadcast(rstd_b[:], rsrc)
            nmr_b = self.p_bc.tile([P, ntok], F32, tag="bcast", bufs=4,
                                   name=f"nmrb{pi}")
            nc.gpsimd.partition_broadcast(nmr_b[:], nsrc)
            outs.append((rstd_b, nmr_b))
        return outs[0], (outs[1] if both else None)

    def ln_apply_B(self, xs, rstd_b, nmr_b, ntok, out_dtype=BF16, out_pool=None,
                   out_tag="xn", gb=None):
        nc = self.nc
        out_pool = out_pool or self.p_xn
        outs = []
        for c in range(NCH):
            t1 = self.p_misc.tile([P, ntok], F32, tag="sq", bufs=5)
            nc.vector.tensor_tensor(out=t1[:], in0=xs[c][:], in1=rstd_b[:],
                                    op=OP.mult)
            if gb is None:
                o = out_pool.tile([P, ntok], out_dtype, tag=out_tag)
                nc.gpsimd.tensor_tensor(out=o[:], in0=t1[:], in1=nmr_b[:],
                                        op=OP.add)
            else:
                t2 = self.p_misc.tile([P, ntok], F32, tag="sq", bufs=5)
                nc.gpsimd.tensor_tensor(out=t2[:], in0=t1[:], in1=nmr_b[:],
                                        op=OP.add)
                o = out_pool.tile([P, ntok], out_dtype, tag=out_tag)
                g_t, b_t = gb
                nc.scalar.activation(o[:], t2[:], AF.Identity,
                                     scale=g_t[:, c:c + 1], bias=b_t[:, c:c + 1])
            outs.append(o)
        return outs


def build_nc(debug=False):
    nc = bacc.Bacc("TRN2", target_bir_lowering=False, debug=False)

    din = {}
    def dram_in(name, shape, dtype):
        din[name] = nc.dram_tensor(name, list(shape), dtype,
                                   kind="ExternalInput").ap()

    dram_in("xidx", (TOK, 1), I32)
    dram_in("emb_real", (V, D), F32)
    dram_in("emb_imag", (V, D), F32)
    dram_in("freq", (1, D), F32)
    dram_in("srange", (P, S // P), F32)
    for ln_ in LAYERS:
        for wn in ("q", "k"):
            dram_in(f"{ln_}_{wn}T", (P, NCH, D), BF16)
            dram_in(f"{ln_}_{wn}b", (P, H), F32)
        dram_in(f"{ln_}_vT", (P, NCH, D), BF16)
        dram_in(f"{ln_}_vbrow", (1, D), F32)
        dram_in(f"{ln_}_oT", (P, NCH, D), BF16)
        dram_in(f"{ln_}_ob", (P, NCH), F32)
    for nm in ("wp", "ps", "rd"):
        dram_in(f"{nm}T", (P, NCH, D), BF16)
        dram_in(f"{nm}b", (P, NCH), F32)
    dram_in("memT", (P, NCH, D), BF16)
    dram_in("mem_r", (P, NCH, D), BF16)
    dram_in("mem_i", (P, NCH, D), BF16)
    for nm in ("ln_model_g", "ln_model_b", "ln_hier_g", "ln_hier_b"):
        dram_in(nm, (P, NCH), F32)
    dram_in("cwT", (P, NVT, D), BF16)
    dram_in("cb", (P, NVT), F32)

    out_dram = nc.dram_tensor("out", [V, TOK], BF16, kind="ExternalOutput").ap()
    taps = {}
    if debug:
        for nm in ("dbg_hr", "dbg_hi", "dbg_wordr", "dbg_wordi",
                   "dbg_fusedr", "dbg_fusedi", "dbg_density"):
            taps[nm] = nc.dram_tensor(nm, [D, TOK], F32,
                                      kind="ExternalOutput").ap()
        for nm in ("dbg_xnr", "dbg_rstdb", "dbg_q0", "dbg_k0", "dbg_vt0",
                   "dbg_exp0", "dbg_rb0", "dbg_attnr0", "dbg_av0"):
            taps[nm] = nc.dram_tensor(nm, [P, TOK], F32,
                                      kind="ExternalOutput").ap()

    with tile.TileContext(nc) as tc:
        with ExitStack() as ctx:
            k = K(ctx, tc)
            _build_body(k, din, out_dram, taps)
    nc.compile()
    return nc


def _tap(k, taps, name, xs):
    if name in taps:
        for c in range(NCH):
            k.nc.sync.dma_start(taps[name][P * c:P * (c + 1), :],
                                xs[c][:].bitcast(F32))


def _build_body(k, din, out_dram, taps):
    nc = k.nc

    # ---- stage A: embedding + positional + initial cln (token-major) ----
    idx = k.p_const.tile([P, NCH], I32, tag="idx")
    nc.sync.dma_start(idx[:], din["xidx"][:, 0].rearrange("(t p) -> p t", p=P))

    freq_row = k.p_const.tile([1, D], F32, tag="freqr")
    nc.sync.dma_start(freq_row[:], din["freq"][:])
    freq_b = k.p_const.tile([P, D], F32, tag="freqb")
    nc.gpsimd.partition_broadcast(freq_b[:], freq_row[:])
    srange = k.p_const.tile([P, S // P], F32, tag="srange")
    nc.sync.dma_start(srange[:], din["srange"][:])

    pe = {}
    for j in range(S // P):
        ang = k.p_misc.tile([P, D], F32, tag="sq")
        nc.vector.tensor_scalar(out=ang[:], in0=freq_b[:],
                                scalar1=srange[:, j:j + 1], scalar2=None,
                                op0=OP.mult)
        pr = k.p_misc.tile([P, D], F32, tag="pe")
        nc.scalar.activation(pr[:], ang[:], AF.Sin, bias=k.c_pihalf[:])
        pi = k.p_misc.tile([P, D], F32, tag="pe")
        nc.scalar.activation(pi[:], ang[:], AF.Sin)
        pe[j] = {"r": pr, "i": pi}

    g_model = k.p_const.tile([P, NCH], F32, tag="gmod")
    nc.sync.dma_start(g_model[:], din["ln_model_g"][:])
    b_model = k.p_const.tile([P, NCH], F32, tag="bmod")
    nc.sync.dma_start(b_model[:], din["ln_model_b"][:])
    g_hier = k.p_const.tile([P, NCH], F32, tag="ghier")
    nc.sync.dma_start(g_hier[:], din["ln_hier_g"][:])
    b_hier = k.p_const.tile([P, NCH], F32, tag="bhier")
    nc.sync.dma_start(b_hier[:], din["ln_hier_b"][:])

    h_B = {"r": [], "i": []}
    for part in ("r", "i"):
        for c in range(NCH):
            h_B[part].append(k.p_s512.tile([P, TOK], F32R, tag=f"st512{part}", name=f"hB{part}{c}"))

    for t in range(TOK // P):
        for part, tbl in (("r", "emb_real"), ("i", "emb_imag")):
            emb = k.p_misc.tile([P, D], F32, tag="sq")
            nc.gpsimd.indirect_dma_start(
                out=emb[:], out_offset=None, in_=din[tbl][:],
                in_offset=bass.IndirectOffsetOnAxis(ap=idx[:, t:t + 1], axis=0))
            hh = k.p_misc.tile([P, D], F32, tag="htm")
            nc.vector.tensor_tensor(out=hh[:], in0=emb[:],
                                    in1=pe[t % 2][part][:], op=OP.add)
            ssum = k.p_row.tile([P, 1], F32, tag="cst")
            nc.vector.reduce_sum(ssum[:], hh[:], AX.X)
            sqscr = k.p_misc.tile([P, D], F32, tag="sq")
            ssq = k.p_row.tile([P, 1], F32, tag="cst")
            nc.scalar.activation(sqscr[:], hh[:], AF.Square, accum_out=ssq[:])
            mean = k.p_row.tile([P, 1], F32, tag="cst")
            nc.vector.tensor_scalar(out=mean[:], in0=ssum[:], scalar1=1.0 / D,
                                    scalar2=None, op0=OP.mult)
            m2 = k.p_row.tile([P, 1], F32, tag="cst")
            nc.vector.tensor_tensor(out=m2[:], in0=mean[:], in1=mean[:],
                                    op=OP.mult)
            var = k.p_row.tile([P, 1], F32, tag="cst")
            nc.vector.scalar_tensor_tensor(out=var[:], in0=ssq[:],
                                           scalar=1.0 / D, in1=m2[:],
                                           op0=OP.mult, op1=OP.subtract)
            lnv = k.p_row.tile([P, 1], F32, tag="cst")
            nc.scalar.activation(lnv[:], var[:], AF.Ln, bias=k.c_eps[:])
            rstd = k.p_row.tile([P, 1], F32, tag="cst")
            nc.scalar.activation(rstd[:], lnv[:], AF.Exp, scale=-0.5)
            nmr = k.p_row.tile([P, 1], F32, tag="cst")
            nc.vector.scalar_tensor_tensor(out=nmr[:], in0=mean[:], scalar=-1.0,
                                           in1=rstd[:], op0=OP.mult, op1=OP.mult)
            xn = k.p_misc.tile([P, D], F32, tag="htm")
            nc.scalar.activation(xn[:], hh[:], AF.Identity, scale=rstd[:],
                                 bias=nmr[:])
            for c in range(NCH):
                pst = k.p_pst.tile([P, P], F32, tag="pst", space="PSUM")
                nc.tensor.transpose(pst[:], xn[:, P * c:P * (c + 1)], k.ident[:])
                nc.scalar.activation(h_B[part][c][:, P * t:P * (t + 1)], pst[:],
                                     AF.Identity, scale=g_model[:, c:c + 1],
                                     bias=b_model[:, c:c + 1])

    _tap(k, taps, "dbg_hr", h_B["r"])
    _tap(k, taps, "dbg_hi", h_B["i"])

    # ---- attention layer ----
    def attn_layer(prefix, xs_r, xs_i, ntok, seqlen, spool):
        n_kch = seqlen // P              # ki chunks per sequence (2/1/1)
        nsb = seqlen // P

        (rstd_br, nmr_br), ri = k.ln_stats2_B(xs_r, xs_i, ntok)
        xn_r = k.ln_apply_B(xs_r, rstd_br, nmr_br, ntok)
        xn_i = k.ln_apply_B(xs_i, ri[0], ri[1], ntok)

        wq = k.load_w(din[f"{prefix}_qT"])
        wk = k.load_w(din[f"{prefix}_kT"])
        wv = k.load_w(din[f"{prefix}_vT"])
        wo = k.load_w(din[f"{prefix}_oT"])
        qb = k.p_w.tile([P, H], F32, tag="hb", bufs=6)
        nc.sync.dma_start(qb[:], din[f"{prefix}_qb"][:])
        kb = k.p_w.tile([P, H], F32, tag="hb", bufs=6)
        nc.sync.dma_start(kb[:], din[f"{prefix}_kb"][:])
        ob = k.p_w.tile([P, NCH], F32, tag="hb", bufs=6)
        nc.sync.dma_start(ob[:], din[f"{prefix}_ob"][:])
        vb_row = k.p_row.tile([1, D], F32, tag="vbrow", bufs=2)
        nc.sync.dma_start(vb_row[:], din[f"{prefix}_vbrow"][:])
        vb_b = k.p_bc.tile([P, D], F32, tag="vbb", bufs=2)
        nc.gpsimd.partition_broadcast(vb_b[:], vb_row[:])

        def qk_cat(w, bias):
            tiles = []
            for h in range(H):
                ps = k.p_ps.tile([P, ntok], F32, tag="ps", space="PSUM")
                for c in range(NCH):
                    nc.tensor.matmul(
                        ps[0:64, :], k.wslice(w, c)[:, 64 * h:64 * h + 64],
                        xn_r[c][:], start=(c == 0), stop=(c == NCH - 1),
                        tile_position=(0, 0))
                for c in range(NCH):
                    nc.tensor.matmul(
                        ps[64:128, :], k.wslice(w, c)[:, 64 * h:64 * h + 64],
                        xn_i[c][:], start=(c == 0), stop=(c == NCH - 1),
                        tile_position=(0, 64))
                t = k.p_qkv.tile([P, ntok], BF16, tag="qkcat", bufs=18)
                nc.vector.tensor_scalar(out=t[:], in0=ps[:],
                                        scalar1=bias[:, h:h + 1], scalar2=None,
                                        op0=OP.add)
                tiles.append(t)
            return tiles

        q_cat = qk_cat(wq, qb)
        k_cat = qk_cat(wk, kb)

        vT = {"r": [], "i": []}
        for part, xn in (("r", xn_r), ("i", xn_i)):
            for m in range(ntok // P):
                ps = k.p_ps.tile([P, D], F32, tag="ps", space="PSUM")
                for c in range(NCH):
                    nc.tensor.matmul(ps[:], xn[c][:, P * m:P * (m + 1)],
                                     k.wslice(wv, c), start=(c == 0),
                                     stop=(c == NCH - 1))
                t = k.p_qkv.tile([P, D], BF16, tag="vt", bufs=8)
                nc.vector.tensor_tensor(out=t[:], in0=ps[:], in1=vb_b[:],
                                        op=OP.add)
                vT[part].append(t)

        attn_s = {"r": [], "i": []}
        for c in range(NCH):
            attn_s["r"].append(k.p_attn.tile([P, ntok], BF16, tag="attnr",
                                             bufs=5, name=f"attnr{c}"))
            attn_s["i"].append(k.p_attn.tile([P, ntok], BF16, tag="attni",
                                             bufs=5, name=f"attni{c}"))

        for b in range(BPC):
            sl = slice(seqlen * b, seqlen * (b + 1))
            for h in range(H):
                sums = k.p_psr.tile([1, seqlen], F32, tag="psrow", space="PSUM")
                expT = []
                for j in range(n_kch):
                    pss = k.p_ps.tile([P, seqlen], F32, tag="ps", space="PSUM")
                    nc.tensor.matmul(
                        pss[:],
                        k_cat[h][:, seqlen * b + P * j:seqlen * b + P * (j + 1)],
                        q_cat[h][:, sl], start=True, stop=True)
                    e = k.p_attn.tile([P, seqlen], BF16, tag="expT", bufs=6)
                    nc.scalar.activation(e[:], pss[:], AF.Exp, scale=1.0 / 8.0)
                    expT.append(e)
                    nc.tensor.matmul(sums[:], k.ones16[:, 0:1], e[:],
                                     start=(j == 0), stop=(j == n_kch - 1))
                rr = k.p_misc.tile([1, seqlen], F32, tag="rowpack", bufs=4)
                nc.vector.reciprocal_approx_fast(rr[:], sums[:])
                rb = k.p_attn.tile([P, seqlen], F32, tag="rb", bufs=3)
                nc.gpsimd.partition_broadcast(rb[:], rr[:])

                ps = k.p_ps.tile([P, seqlen], F32, tag="ps", space="PSUM")
                for part, cofs in (("r", 0), ("i", 64)):
                    for j in range(n_kch):
                        nc.tensor.matmul(
                            ps[cofs:cofs + 64, :],
                            vT[part][b * nsb + j][:, 64 * h:64 * h + 64],
                            expT[j][:], start=(j == 0), stop=(j == n_kch - 1),
                            tile_position=(0, cofs))
                cp, half = h // 2, h % 2
                nc.vector.tensor_tensor(
                    out=attn_s["r"][cp][64 * half:64 * half + 64, sl],
                    in0=ps[0:64, :], in1=rb[0:64, :], op=OP.mult)
                nc.vector.tensor_tensor(
                    out=attn_s["i"][cp][64 * half:64 * half + 64, sl],
                    in0=ps[64:128, :], in1=rb[64:128, :], op=OP.mult)

        new_r, new_i = [], []
        for part, attn_t, xs, outl in (("r", attn_s["r"], xs_r, new_r),
                                       ("i", attn_s["i"], xs_i, new_i)):
            for m in range(NCH):
                ps = k.p_ps.tile([P, ntok], F32, tag="ps", space="PSUM")
                for c in range(NCH):
                    nc.tensor.matmul(ps[:], k.wslice(wo, c, m), attn_t[c][:],
                                     start=(c == 0), stop=(c == NCH - 1))
                o = spool.tile([P, ntok], F32R, tag=f"st{ntok}{part}", bufs=8,
                               name=f"res{part}{m}")
                nc.vector.scalar_tensor_tensor(out=o[:], in0=ps[:],
                                               scalar=ob[:, m:m + 1],
                                               in1=xs[m][:], op0=OP.add,
                                               op1=OP.add)
                outl.append(o)
        return new_r, new_i

    def pool2(xs, ntok, dst_pool, part):
        outs = []
        for c in range(NCH):
            o = dst_pool.tile([P, ntok // 2], F32, tag=f"st{ntok // 2}{part}")
            nc.vector.tensor_tensor(out=o[:], in0=xs[c][:, 0:ntok:2],
                                    in1=xs[c][:, 1:ntok:2], op=OP.add)
            nc.vector.tensor_scalar(out=o[:], in0=o[:], scalar1=0.5,
                                    scalar2=None, op0=OP.mult)
            outs.append(o)
        return outs

    def proj_up(prefix, src_r, src_i, base_r, base_i, ntok_half, dst_pool):
        """base + clin_rb(up2(src), proj): bias on real part only."""
        w = k.load_w(din[f"{prefix}T"])
        bsl = k.p_w.tile([P, NCH], F32, tag="hb")
        nc.sync.dma_start(bsl[:], din[f"{prefix}b"][:])
        ntok = ntok_half * 2
        outs = {"r": [], "i": []}
        for part, src, base in (("r", src_r, base_r), ("i", src_i, base_i)):
            src16 = []
            for c in range(NCH):
                t = k.p_xn.tile([P, ntok_half], BF16, tag="xn")
                nc.vector.tensor_copy(t[:], src[c][:])
                src16.append(t)
            for m in range(NCH):
                ps = k.p_ps.tile([P, ntok], F32, tag="ps", space="PSUM")
                for c in range(NCH):
                    rep = src16[c][:].rearrange("p (n o) -> p n o", o=1) \
                        .broadcast_to([P, ntok_half, 2])
                    nc.tensor.matmul(ps[:], k.wslice(w, c, m), rep,
                                     start=(c == 0), stop=(c == NCH - 1))
                o = dst_pool.tile([P, ntok], F32, tag=f"st{ntok}{part}")
                if part == "r":
                    nc.vector.scalar_tensor_tensor(out=o[:], in0=ps[:],
                                                   scalar=bsl[:, m:m + 1],
                                                   in1=base[m][:], op0=OP.add,
                                                   op1=OP.add)
                else:
                    nc.vector.tensor_tensor(out=o[:], in0=ps[:], in1=base[m][:],
                                            op=OP.add)
                outs[part].append(o)
        return outs["r"], outs["i"]

    phr_r = pool2(word_r, TOK, k.p_s256, "r")
    phr_i = pool2(word_i, TOK, k.p_s256, "i")
    for i in range(DEPTH):
        phr_r, phr_i = attn_layer(f"p{i}", phr_r, phr_i, TOK // 2, S // 2,
                                  k.p_s256)

    comb_r, comb_i = proj_up("wp", phr_r, phr_i, word_r, word_i, TOK // 2,
                             k.p_s512)

    sent_r = pool2(comb_r, TOK, k.p_s256, "r")
    sent_i = pool2(comb_i, TOK, k.p_s256, "i")
    for i in range(max(1, DEPTH // 2)):
        sent_r, sent_i = attn_layer(f"s{i}", sent_r, sent_i, TOK // 2, S // 2,
                                    k.p_s256)

    pre_r, pre_i = proj_up("ps", sent_r, sent_i, comb_r, comb_i, TOK // 2,
                           k.p_s512)

    # ---- fused cln (full apply with ln_hier g/b) ----
    rstd_b, nmr_b = k.ln_stats_B(pre_r, TOK)
    fused_r = k.ln_apply_B(pre_r, rstd_b, nmr_b, TOK, out_dtype=F32,
                           out_pool=k.p_f32a, out_tag="fusedr",
                           gb=(g_hier, b_hier))
    rstd_b, nmr_b = k.ln_stats_B(pre_i, TOK)
    fused_i = k.ln_apply_B(pre_i, rstd_b, nmr_b, TOK, out_dtype=F32,
                           out_pool=k.p_f32a, out_tag="fusedi",
                           gb=(g_hier, b_hier))
    _tap(k, taps, "dbg_fusedr", fused_r)
    _tap(k, taps, "dbg_fusedi", fused_i)

    # ---- memory read ----
    fused_r16 = []
    for c in range(NCH):
        t = k.p_xn.tile([P, TOK], BF16, tag="xn")
        nc.vector.tensor_copy(t[:], fused_r[c][:])
        fused_r16.append(t)

    w_memT = k.load_w(din["memT"])
    w_memr = k.load_w(din["mem_r"])
    w_memi = k.load_w(din["mem_i"])

    nsq = []
    for c in range(NCH):
        s1 = k.p_misc.tile([P, TOK], F32, tag="sq")
        nc.scalar.activation(s1[:], fused_r[c][:], AF.Square)
        s2 = k.p_misc.tile([P, TOK], F32, tag="sq")
        nc.scalar.activation(s2[:], fused_i[c][:], AF.Square)
        t = k.p_misc.tile([P, TOK], F32, tag="nsq")
        nc.vector.tensor_tensor(out=t[:], in0=s1[:], in1=s2[:], op=OP.add)
        nsq.append(t)

    wT_mem = []
    for cm in range(NCH):
        wT_mem.append(k.p_qkv.tile([P, TOK], BF16, tag="wTm", name=f"wTm{cm}"))

    for m in range(TOK // P):
        ps_sc = k.p_ps.tile([P, M_MEM], F32, tag="ps", space="PSUM")
        for c in range(NCH):
            nc.tensor.matmul(ps_sc[:], fused_r16[c][:, P * m:P * (m + 1)],
                             k.wslice(w_memT, c), start=(c == 0),
                             stop=(c == NCH - 1))
        nT = k.p_misc.tile([P, D], F32, tag="nT")
        for c in range(NCH):
            pst = k.p_pst.tile([P, P], F32, tag="pst", space="PSUM")
            nc.tensor.transpose(pst[:], nsq[c][:, P * m:P * (m + 1)], k.ident[:])
            nc.scalar.activation(nT[:, P * c:P * (c + 1)], pst[:], AF.Ln)
        nrm = k.p_misc.tile([P, D], F32, tag="nT")
        nc.scalar.activation(nrm[:], nT[:], AF.Exp, scale=0.5)
        nc.vector.tensor_scalar(out=nrm[:], in0=nrm[:], scalar1=1e-8,
                                scalar2=None, op0=OP.add)
        rn = k.p_misc.tile([P, D], F32, tag="nT")
        nc.vector.reciprocal_approx_fast(rn[:], nrm[:])
        z = k.p_misc.tile([P, M_MEM], F32, tag="nT")
        nc.vector.tensor_tensor(out=z[:], in0=ps_sc[:], in1=rn[:], op=OP.mult)
        negmax = k.p_row.tile([P, 1], F32, tag="cst")
        nc.vector.reduce_max(negmax[:], z[:], AX.X, negate=True)
        ez = k.p_misc.tile([P, M_MEM], BF16, tag="ez")
        ssum = k.p_row.tile([P, 1], F32, tag="cst")
        nc.scalar.activation(ez[:], z[:], AF.Exp, bias=negmax[:],
                             accum_out=ssum[:])
        rs = k.p_row.tile([P, 1], F32, tag="cst")
        nc.vector.reciprocal_approx_fast(rs[:], ssum[:])
        wgt = k.p_misc.tile([P, M_MEM], BF16, tag="ez")
        nc.vector.tensor_scalar(out=wgt[:], in0=ez[:], scalar1=rs[:],
                                scalar2=None, op0=OP.mult)
        for cm in range(NCH):
            pst16 = k.p_pst.tile([P, P], BF16, tag="pst", space="PSUM")
            nc.tensor.transpose(pst16[:], wgt[:, P * cm:P * (cm + 1)],
                                k.ident16[:])
            nc.vector.tensor_copy(wT_mem[cm][:, P * m:P * (m + 1)], pst16[:])

    rd_w = k.load_w(din["rdT"])
    rd_b = k.p_w.tile([P, NCH], F32, tag="hb")
    nc.sync.dma_start(rd_b[:], din["rdb"][:])

    h2 = {"r": [], "i": []}
    for part, wmem, fus in (("r", w_memr, fused_r), ("i", w_memi, fused_i)):
        cont16 = []
        for cd in range(NCH):
            ps = k.p_ps.tile([P, TOK], F32, tag="ps", space="PSUM")
            for cm in range(NCH):
                nc.tensor.matmul(ps[:], k.wslice(wmem, cm, cd), wT_mem[cm][:],
                                 start=(cm == 0), stop=(cm == NCH - 1))
            t = k.p_xn.tile([P, TOK], BF16, tag="xn")
            nc.vector.tensor_copy(t[:], ps[:])
            cont16.append(t)
        for m in range(NCH):
            ps = k.p_ps.tile([P, TOK], F32, tag="ps", space="PSUM")
            for c in range(NCH):
                nc.tensor.matmul(ps[:], k.wslice(rd_w, c, m), cont16[c][:],
                                 start=(c == 0), stop=(c == NCH - 1))
            o = k.p_f32a.tile([P, TOK], F32, tag=f"h2{part}")
            nc.vector.scalar_tensor_tensor(out=o[:], in0=ps[:],
                                           scalar=rd_b[:, m:m + 1],
                                           in1=fus[m][:], op0=OP.add, op1=OP.add)
            h2[part].append(o)

    # ---- final cln + density ----
    rstd_b, nmr_b = k.ln_stats_B(h2["r"], TOK)
    hn_r = k.ln_apply_B(h2["r"], rstd_b, nmr_b, TOK, out_dtype=F32,
                        out_pool=k.p_f32a, out_tag="hn")
    rstd_b, nmr_b = k.ln_stats_B(h2["i"], TOK)
    hn_i = k.ln_apply_B(h2["i"], rstd_b, nmr_b, TOK, out_dtype=F32,
                        out_pool=k.p_f32a, out_tag="hn")
    density16 = []
    for c in range(NCH):
        d1 = k.p_misc.tile([P, TOK], F32, tag="sq")
        nc.scalar.activation(d1[:], hn_r[c][:], AF.Square,
                             scale=g_model[:, c:c + 1], bias=b_model[:, c:c + 1])
        d2 = k.p_misc.tile([P, TOK], F32, tag="sq")
        nc.scalar.activation(d2[:], hn_i[c][:], AF.Square,
                             scale=g_model[:, c:c + 1], bias=b_model[:, c:c + 1])
        dt_ = k.p_misc.tile([P, TOK], BF16, tag="dens")
        nc.vector.tensor_tensor(out=dt_[:], in0=d1[:], in1=d2[:], op=OP.add)
        density16.append(dt_)
    if "dbg_density" in taps:
        for c in range(NCH):
            f32t = k.p_misc.tile([P, TOK], F32, tag="sq")
            nc.vector.tensor_copy(f32t[:], density16[c][:])
            nc.sync.dma_start(taps["dbg_density"][P * c:P * (c + 1), :], f32t[:])

    # ---- collapse ----
    cb_t = k.p_const.tile([P, NVT], F32, tag="cb")
    nc.sync.dma_start(cb_t[:], din["cb"][:])
    for t in range(NVT):
        cw = k.p_w.tile([P, D], BF16, tag="cw")
        nc.sync.dma_start(cw[:], din["cwT"][:, t, :])
        ps = k.p_ps.tile([P, TOK], F32, tag="ps", space="PSUM")
        for c in range(NCH):
            nc.tensor.matmul(ps[:], cw[:, P * c:P * (c + 1)], density16[c][:],
                             start=(c == 0), stop=(c == NCH - 1))
        o = k.p_out.tile([P, TOK], BF16, tag="out")
        nc.scalar.activation(o[:], ps[:], AF.Relu, bias=cb_t[:, t:t + 1])
        nc.sync.dma_start(out_dram[P * t:P * (t + 1), :], o[:])


# ----------------------------------------------------------------------------
# entry point
# ----------------------------------------------------------------------------

_RUN_KW = {}


def kernel(x=None, params=None, **kw):
    if x is None:
        x = kw.pop("x")
    if params is None:
        params = kw.pop("params")
    in_maps = _prep(x, params)
    nc = build_nc(debug=DEBUG)
    res = run_bass_kernel_spmd(nc, in_maps, core_ids=list(range(NCORES)),
                               **_RUN_KW)
    outs = []
    for c in range(NCORES):
        o = np.asarray(res.results[c]["out"])          # [V, TOK] bf16
        o = o.reshape(V, BPC, S).transpose(1, 2, 0).astype(np.float32)
        outs.append(o)
    full = np.concatenate(outs, axis=0)                # [B, S, V]
    kernel.last_results = res
    return full


# revision 29
# speedup vs baseline: 1.0251x; 1.0227x over previous
"""Trainium2 Bass kernel for nn_AdvancedSFIN (hierarchical complex transformer).

Self-contained: builds a single-core Bass/Tile program, runs it SPMD on 8
NeuronCores (data-parallel over batch: 2 sequences per core), reassembles the
full [16, 256, 32000] float32 output on the host.

Design notes:
- Activations live in "layout B": [D(partitions, 4 chunks of 128), tok(free)].
  Linear layers contract over D natively (lhsT = host-pre-transposed weights).
- All matmuls run in bf16 (validated ~3.4e-3 scale-relative error end to end);
  LayerNorm statistics matmuls use float32r (full-rate fp32 path).
- LayerNorm over D (partition axis) uses ones-vector matmuls for sum/sum-sq,
  gpsimd partition_broadcast for the per-token stats, and exp(-0.5*ln(var+eps))
  on the Scalar engine for rsqrt (keeps the natural_log_exp ACT table resident;
  sqrt never appears on ACT).
- Attention per (seq, head): scoresT = k_cat.T @ q_cat with [real|imag]
  concatenated on the contraction axis; softmax over the partition (ki) axis
  via ones-matmul sums (scores are bounded ~|1.7| so no max subtraction);
  AV uses token-major V (produced directly by using the normalized input as
  the stationary matmul operand), real/imag packed into one PSUM tile by
  column tiling; the 1/sum normalization rides on PSUM eviction.
- ln_hier gamma/beta are folded into the QKV weights on the host (the cln
  ahead of each attention feeds only those linears).
- Memory read + collapse produce logits v-major [V, tok] so the collapse bias
  + relu(+1e-10 folded into the bias) ride per-partition on ScalarE; the host
  transposes back and upcasts bf16 -> f32.
"""

import numpy as np
import ml_dtypes
from contextlib import ExitStack

import concourse.bass as bass
import concourse.bacc as bacc
import concourse.tile as tile
from concourse import mybir
from concourse.bass_utils import run_bass_kernel_spmd
from concourse.masks import make_identity

P = 128
V, D, H, M_MEM = 32000, 512, 8, 512
B, S = 16, 256
DEPTH = 2
NCORES = 8
BPC = B // NCORES          # sequences per core
TOK = BPC * S              # tokens per core (512)
HD = D // H                # head dim (64)
NCH = D // P               # d-chunks (4)
NVT = V // P               # collapse v-tiles (250)
EPS = 1e-5

F32 = mybir.dt.float32
F32R = mybir.dt.float32r
BF16 = mybir.dt.bfloat16
I32 = mybir.dt.int32
AF = mybir.ActivationFunctionType
OP = mybir.AluOpType
AX = mybir.AxisListType

BF = ml_dtypes.bfloat16

DEBUG = False  # when True, adds DRAM taps for intermediates (dev only)

LAYERS = [f"w{i}" for i in range(DEPTH)] + \
         [f"p{i}" for i in range(DEPTH)] + \
         [f"s{i}" for i in range(max(1, DEPTH // 2))]


# ----------------------------------------------------------------------------
# host-side prep
# ----------------------------------------------------------------------------

def _wt_tiled(w_t: np.ndarray) -> np.ndarray:
    """[din, dout] -> [128, din//128, dout] bf16 (k-chunk c at [:, c, :])."""
    din, dout = w_t.shape
    return np.ascontiguousarray(
        w_t.reshape(din // P, P, dout).transpose(1, 0, 2)).astype(BF)


def _bias_pp(b: np.ndarray) -> np.ndarray:
    """[512] -> [128, 4] f32 per-partition layout (chunk c in column c)."""
    return np.ascontiguousarray(b.reshape(-1, P).T).astype(np.float32)


def _prep(x: np.ndarray, params: dict):
    gh = np.asarray(params["ln_hier_g"], np.float32)
    bh = np.asarray(params["ln_hier_b"], np.float32)

    shared = {}
    shared["emb_real"] = np.ascontiguousarray(np.asarray(params["emb_real"], np.float32))
    shared["emb_imag"] = np.ascontiguousarray(np.asarray(params["emb_imag"], np.float32))
    shared["freq"] = np.ascontiguousarray(
        np.asarray(params["freq"], np.float32).reshape(1, D))
    sr = np.arange(S, dtype=np.float32).reshape(S // P, P).T
    shared["srange"] = np.ascontiguousarray(sr)

    def attn_prep(prefix, p):
        for wname in ("q", "k"):
            w = np.asarray(p[wname]["W"], np.float32)
            b = np.asarray(p[wname]["b"], np.float32)
            w_eff = w * gh[None, :]
            b_eff = b + w @ bh
            shared[f"{prefix}_{wname}T"] = _wt_tiled(w_eff.T)
            bb = np.empty((P, H), np.float32)
            for h in range(H):
                bb[0:64, h] = b_eff[64 * h:64 * h + 64]
                bb[64:128, h] = b_eff[64 * h:64 * h + 64]
            shared[f"{prefix}_{wname}b"] = np.ascontiguousarray(bb)
        w = np.asarray(p["v"]["W"], np.float32)
        b = np.asarray(p["v"]["b"], np.float32)
        shared[f"{prefix}_vT"] = _wt_tiled((w * gh[None, :]).T)
        shared[f"{prefix}_vbrow"] = np.ascontiguousarray(
            (b + w @ bh).reshape(1, D)).astype(np.float32)
        w = np.asarray(p["o"]["W"], np.float32)
        b = np.asarray(p["o"]["b"], np.float32)
        shared[f"{prefix}_oT"] = _wt_tiled(w.T)
        shared[f"{prefix}_ob"] = _bias_pp(b)

    for prefix, p in zip(LAYERS, list(params["word"]) + list(params["phrase"])
                         + list(params["sentence"])):
        attn_prep(prefix, p)

    for nm, key in (("wp", "proj_wp"), ("ps", "proj_ps"), ("rd", "read")):
        w = np.asarray(params[key]["W"], np.float32)
        b = np.asarray(params[key]["b"], np.float32)
        shared[f"{nm}T"] = _wt_tiled(w.T)
        shared[f"{nm}b"] = _bias_pp(b)

    mem_r = np.asarray(params["mem_real"], np.float32)
    mem_i = np.asarray(params["mem_imag"], np.float32)
    shared["memT"] = _wt_tiled(mem_r.T)
    shared["mem_r"] = _wt_tiled(mem_r)
    shared["mem_i"] = _wt_tiled(mem_i)

    shared["ln_model_g"] = _bias_pp(np.asarray(params["ln_model_g"], np.float32))
    shared["ln_model_b"] = _bias_pp(np.asarray(params["ln_model_b"], np.float32))
    shared["ln_hier_g"] = _bias_pp(gh)
    shared["ln_hier_b"] = _bias_pp(bh)

    cw = np.asarray(params["collapse_W"], np.float32)   # [V, D]
    cb = np.asarray(params["collapse_b"], np.float32)
    A = cw.reshape(NVT, P, NCH, P).transpose(3, 0, 2, 1)   # [p, t, c, v']
    shared["cwT"] = np.ascontiguousarray(A.reshape(P, NVT, D)).astype(BF)
    shared["cb"] = np.ascontiguousarray(
        (cb + 1e-10).reshape(NVT, P).T).astype(np.float32)

    x = np.asarray(x).astype(np.int32)
    per_core = []
    for c in range(NCORES):
        m = dict(shared)
        m["xidx"] = np.ascontiguousarray(x[BPC * c:BPC * (c + 1)].reshape(TOK, 1))
        per_core.append(m)
    return per_core


# ----------------------------------------------------------------------------
# device program
# ----------------------------------------------------------------------------

class K:
    def __init__(self, ctx, tc):
        self.ctx = ctx
        self.tc = tc
        self.nc = tc.nc
        nc = self.nc
        ep = ctx.enter_context

        self.p_const = ep(tc.tile_pool(name="const", bufs=1))
        self.p_s512 = ep(tc.tile_pool(name="s512", bufs=8))    # residual streams
        self.p_misc = ep(tc.tile_pool(name="misc", bufs=6))
        self.p_bc = ep(tc.tile_pool(name="bcst", bufs=4))
        self.p_row = ep(tc.tile_pool(name="rows", bufs=4))
        self.p_ps = ep(tc.tile_pool(name="ps", bufs=6, space="PSUM"))
        self.p_psr = self.p_ps
        self.p_pst = ep(tc.tile_pool(name="pst", bufs=1, space="PSUM"))
        # scope-dependent pools, assigned by _build_body:
        self.p_s256 = None
        self.p_f32a = None
        self.p_xn = None
        self.p_qkv = None
        self.p_attn = None
        self.p_w = None
        self.p_out = None

        self.ident = self.p_const.tile([P, P], F32, tag="identf")
        make_identity(nc, self.ident[:])
        self.ident16 = self.p_const.tile([P, P], BF16, tag="identb")
        nc.vector.tensor_copy(self.ident16[:], self.ident[:])
        ones32 = self.p_const.tile([P, 1], F32, tag="ones32")
        nc.vector.memset(ones32[:], 1.0)
        self.ones = self.p_const.tile([P, 1], F32R, tag="ones")
        nc.vector.tensor_copy(self.ones[:], ones32[:])
        self.ones16 = self.p_const.tile([P, 1], BF16, tag="ones16")
        nc.vector.memset(self.ones16[:], 1.0)
        self.c_pihalf = self.p_const.tile([P, 1], F32, tag="cpih")
        nc.vector.memset(self.c_pihalf[:], float(np.pi / 2))
        self.c_eps = self.p_const.tile([P, 1], F32, tag="ceps")
        nc.vector.memset(self.c_eps[:], EPS)
        self.mask_top = self.p_const.tile([1, P], F32, tag="mtop")
        nc.vector.memset(self.mask_top[:], 0.0)
        nc.vector.memset(self.mask_top[0:1, 0:64], 1.0)
        self.mask_bot = self.p_const.tile([1, P], F32, tag="mbot")
        nc.vector.memset(self.mask_bot[:], 0.0)
        nc.vector.memset(self.mask_bot[0:1, 64:128], 1.0)

    def load_w(self, dram):
        t = self.p_w.tile([P, NCH * D], BF16, tag="wt")
        self.nc.sync.dma_start(t[:], dram.rearrange("p c d -> p (c d)"))
        return t

    def wslice(self, w, c, m=None, width=P):
        if m is None:
            return w[:, D * c:D * (c + 1)]
        return w[:, D * c + P * m: D * c + P * m + width]

    def ln_stats_B(self, xs, ntok):
        """Single-part wrapper around ln_stats2_B."""
        (rb, nb), _ = self.ln_stats2_B(xs, None, ntok)
        return rb, nb

    def ln_stats2_B(self, xs_r, xs_i, ntok):
        """LN stats over D for one or two layout-B tile sets. The Ln/Exp
        rstd chain runs once for both parts via a stride-32 partition AP."""
        nc = self.nc
        both = xs_i is not None
        nparts = 2 if both else 1
        rows = 33 if both else 1
        parts = [xs_r] + ([xs_i] if both else [])
        ps_sums = []
        for xs in parts:
            ps_s = self.p_ps.tile([1, ntok], F32, tag="ps", space="PSUM",
                                  name="ps_s")
            for c in range(NCH):
                nc.tensor.matmul(ps_s[:], self.ones[:, 0:1], xs[c][:],
                                 start=(c == 0), stop=(c == NCH - 1))
            ps_q = self.p_ps.tile([1, ntok], F32, tag="ps", space="PSUM",
                                  name="ps_q")
            for c in range(NCH):
                sq = self.p_misc.tile([P, ntok], F32R, tag="sq", bufs=5)
                nc.scalar.activation(sq[:], xs[c][:], AF.Square)
                nc.tensor.matmul(ps_q[:], self.ones[:, 0:1], sq[:],
                                 start=(c == 0), stop=(c == NCH - 1))
            ps_sums.append((ps_s, ps_q))

        mean = self.p_row.tile([rows, ntok], F32, tag="row", bufs=4)
        var = self.p_row.tile([rows, ntok], F32, tag="row", bufs=4)
        if both:
            nc.vector.memset(mean[:], 1.0)
            nc.vector.memset(var[:], 1.0)
        for pi, (ps_s, ps_q) in enumerate(ps_sums):
            sl = slice(32 * pi, 32 * pi + 1)
            nc.vector.tensor_scalar(out=mean[sl, :], in0=ps_s[:],
                                    scalar1=1.0 / D, scalar2=None, op0=OP.mult)
            m2 = self.p_row.tile([1, ntok], F32, tag="row", bufs=4)
            nc.vector.tensor_tensor(out=m2[:], in0=mean[sl, :],
                                    in1=mean[sl, :], op=OP.mult)
            nc.vector.scalar_tensor_tensor(out=var[sl, :], in0=ps_q[:],
                                           scalar=1.0 / D, in1=m2[:],
                                           op0=OP.mult, op1=OP.subtract)
        def sv(t):
            return t[:]
        lnv = self.p_row.tile([rows, ntok], F32, tag="row", bufs=4)
        nc.scalar.activation(sv(lnv), sv(var), AF.Ln,
                             bias=self.c_eps[0:rows, :])
        rstd = self.p_row.tile([rows, ntok], F32, tag="row", bufs=4)
        nc.scalar.activation(sv(rstd), sv(lnv), AF.Exp, scale=-0.5)
        nmr = self.p_row.tile([rows, ntok], F32, tag="row", bufs=4)
        nc.vector.scalar_tensor_tensor(out=sv(nmr), in0=sv(mean), scalar=-1.0,
                                       in1=sv(rstd), op0=OP.mult, op1=OP.mult)
        outs = []
        for pi in range(nparts):
            sl = slice(32 * pi, 32 * pi + 1)
            # partition_broadcast reads absolute partition 0 -> copy row 32
            # down to a base-0 tile first.
            if pi == 0:
                rsrc, nsrc = rstd[sl, :], nmr[sl, :]
            else:
                r0 = self.p_row.tile([1, ntok], F32, tag="row", bufs=4)
                nc.vector.tensor_copy(r0[:], rstd[sl, :])
                n0 = self.p_row.tile([1, ntok], F32, tag="row", bufs=4)
                nc.vector.tensor_copy(n0[:], nmr[sl, :])
                rsrc, nsrc = r0[:], n0[:]
            rstd_b = self.p_bc.tile([P, ntok], F32, tag="bcast", bufs=4,
                                    name=f"rstdb{pi}")
            nc.gpsimd.partition_broadcast(rstd_b[:], rsrc)
            nmr_b = self.p_bc.tile([P, ntok], F32, tag="bcast", bufs=4,
                                   name=f"nmrb{pi}")
            nc.gpsimd.partition_broadcast(nmr_b[:], nsrc)
            outs.append((rstd_b, nmr_b))
        return outs[0], (outs[1] if both else None)

    def ln_apply_B(self, xs, rstd_b, nmr_b, ntok, out_dtype=BF16, out_pool=None,
                   out_tag="xn", gb=None):
        nc = self.nc
        out_pool = out_pool or self.p_xn
        outs = []
        for c in range(NCH):
            t1 = self.p_misc.tile([P, ntok], F32, tag="sq", bufs=5)
            nc.vector.tensor_tensor(out=t1[:], in0=xs[c][:], in1=rstd_b[:],
                                    op=OP.mult)
            if gb is None:
                o = out_pool.tile([P, ntok], out_dtype, tag=out_tag)
                nc.gpsimd.tensor_tensor(out=o[:], in0=t1[:], in1=nmr_b[:],
                                        op=OP.add)
            else:
                t2 = self.p_misc.tile([P, ntok], F32, tag="sq", bufs=5)
                nc.gpsimd.tensor_tensor(out=t2[:], in0=t1[:], in1=nmr_b[:],
                                        op=OP.add)
                o = out_pool.tile([P, ntok], out_dtype, tag=out_tag)
                g_t, b_t = gb
                nc.scalar.activation(o[:], t2[:], AF.Identity,
                                     scale=g_t[:, c:c + 1], bias=b_t[:, c:c + 1])
            outs.append(o)
        return outs


def build_nc(debug=False):
    nc = bacc.Bacc("TRN2", target_bir_lowering=False, debug=False)

    din = {}
    def dram_in(name, shape, dtype):
        din[name] = nc.dram_tensor(name, list(shape), dtype,
                                   kind="ExternalInput").ap()

    dram_in("xidx", (TOK, 1), I32)
    dram_in("emb_real", (V, D), F32)
    dram_in("emb_imag", (V, D), F32)
    dram_in("freq", (1, D), F32)
    dram_in("srange", (P, S // P), F32)
    for ln_ in LAYERS:
        for wn in ("q", "k"):
            dram_in(f"{ln_}_{wn}T", (P, NCH, D), BF16)
            dram_in(f"{ln_}_{wn}b", (P, H), F32)
        dram_in(f"{ln_}_vT", (P, NCH, D), BF16)
        dram_in(f"{ln_}_vbrow", (1, D), F32)
        dram_in(f"{ln_}_oT", (P, NCH, D), BF16)
        dram_in(f"{ln_}_ob", (P, NCH), F32)
    for nm in ("wp", "ps", "rd"):
        dram_in(f"{nm}T", (P, NCH, D), BF16)
        dram_in(f"{nm}b", (P, NCH), F32)
    dram_in("memT", (P, NCH, D), BF16)
    dram_in("mem_r", (P, NCH, D), BF16)
    dram_in("mem_i", (P, NCH, D), BF16)
    for nm in ("ln_model_g", "ln_model_b", "ln_hier_g", "ln_hier_b"):
        dram_in(nm, (P, NCH), F32)
    dram_in("cwT", (P, NVT, D), BF16)
    dram_in("cb", (P, NVT), F32)

    out_dram = nc.dram_tensor("out", [V, TOK], BF16, kind="ExternalOutput").ap()
    taps = {}
    if debug:
        for nm in ("dbg_hr", "dbg_hi", "dbg_wordr", "dbg_wordi",
                   "dbg_fusedr", "dbg_fusedi", "dbg_density"):
            taps[nm] = nc.dram_tensor(nm, [D, TOK], F32,
                                      kind="ExternalOutput").ap()
        for nm in ("dbg_xnr", "dbg_rstdb", "dbg_q0", "dbg_k0", "dbg_vt0",
                   "dbg_exp0", "dbg_rb0", "dbg_attnr0", "dbg_av0"):
            taps[nm] = nc.dram_tensor(nm, [P, TOK], F32,
                                      kind="ExternalOutput").ap()

    with tile.TileContext(nc) as tc:
        with ExitStack() as ctx:
            k = K(ctx, tc)
            _build_body(k, din, out_dram, taps)
    nc.compile()
    return nc


def _tap(k, taps, name, xs):
    if name in taps:
        for c in range(NCH):
            k.nc.sync.dma_start(taps[name][P * c:P * (c + 1), :],
                                xs[c][:].bitcast(F32))


def _build_body(k, din, out_dram, taps):
    nc = k.nc

    # ---- stage A: embedding + positional + initial cln (token-major) ----
    idx = k.p_const.tile([P, NCH], I32, tag="idx")
    nc.sync.dma_start(idx[:], din["xidx"][:, 0].rearrange("(t p) -> p t", p=P))

    freq_row = k.p_const.tile([1, D], F32, tag="freqr")
    nc.sync.dma_start(freq_row[:], din["freq"][:])
    freq_b = k.p_const.tile([P, D], F32, tag="freqb")
    nc.gpsimd.partition_broadcast(freq_b[:], freq_row[:])
    srange = k.p_const.tile([P, S // P], F32, tag="srange")
    nc.sync.dma_start(srange[:], din["srange"][:])

    pe = {}
    for j in range(S // P):
        ang = k.p_misc.tile([P, D], F32, tag="sq")
        nc.vector.tensor_scalar(out=ang[:], in0=freq_b[:],
                                scalar1=srange[:, j:j + 1], scalar2=None,
                                op0=OP.mult)
        pr = k.p_misc.tile([P, D], F32, tag="pe")
        nc.scalar.activation(pr[:], ang[:], AF.Sin, bias=k.c_pihalf[:])
        pi = k.p_misc.tile([P, D], F32, tag="pe")
        nc.scalar.activation(pi[:], ang[:], AF.Sin)
        pe[j] = {"r": pr, "i": pi}

    g_model = k.p_const.tile([P, NCH], F32, tag="gmod")
    nc.sync.dma_start(g_model[:], din["ln_model_g"][:])
    b_model = k.p_const.tile([P, NCH], F32, tag="bmod")
    nc.sync.dma_start(b_model[:], din["ln_model_b"][:])
    g_hier = k.p_const.tile([P, NCH], F32, tag="ghier")
    nc.sync.dma_start(g_hier[:], din["ln_hier_g"][:])
    b_hier = k.p_const.tile([P, NCH], F32, tag="bhier")
    nc.sync.dma_start(b_hier[:], din["ln_hier_b"][:])

    h_B = {"r": [], "i": []}
    for part in ("r", "i"):
        for c in range(NCH):
            h_B[part].append(k.p_s512.tile([P, TOK], F32R, tag=f"st512{part}", name=f"hB{part}{c}"))

    for t in range(TOK // P):
        for part, tbl in (("r", "emb_real"), ("i", "emb_imag")):
            emb = k.p_misc.tile([P, D], F32, tag="sq")
            nc.gpsimd.indirect_dma_start(
                out=emb[:], out_offset=None, in_=din[tbl][:],
                in_offset=bass.IndirectOffsetOnAxis(ap=idx[:, t:t + 1], axis=0))
            hh = k.p_misc.tile([P, D], F32, tag="htm")
            nc.vector.tensor_tensor(out=hh[:], in0=emb[:],
                                    in1=pe[t % 2][part][:], op=OP.add)
            ssum = k.p_row.tile([P, 1], F32, tag="cst")
            nc.vector.reduce_sum(ssum[:], hh[:], AX.X)
            sqscr = k.p_misc.tile([P, D], F32, tag="sq")
            ssq = k.p_row.tile([P, 1], F32, tag="cst")
            nc.scalar.activation(sqscr[:], hh[:], AF.Square, accum_out=ssq[:])
            mean = k.p_row.tile([P, 1], F32, tag="cst")
            nc.vector.tensor_scalar(out=mean[:], in0=ssum[:], scalar1=1.0 / D,
                                    scalar2=None, op0=OP.mult)
            m2 = k.p_row.tile([P, 1], F32, tag="cst")
            nc.vector.tensor_tensor(out=m2[:], in0=mean[:], in1=mean[:],
                                    op=OP.mult)
            var = k.p_row.tile([P, 1], F32, tag="cst")
            nc.vector.scalar_tensor_tensor(out=var[:], in0=ssq[:],
                                           scalar=1.0 / D, in1=m2[:],
                                           op0=OP.mult, op1=OP.subtract)
            lnv = k.p_row.tile([P, 1], F32, tag="cst")
            nc.scalar.activation(lnv[:], var[:], AF.Ln, bias=k.c_eps[:])
            rstd = k.p_row.tile([P, 1], F32, tag="cst")
            nc.scalar.activation(rstd[:], lnv[:], AF.Exp, scale=-0.5)
            nmr = k.p_row.tile([P, 1], F32, tag="cst")
            nc.vector.scalar_tensor_tensor(out=nmr[:], in0=mean[:], scalar=-1.0,
                                           in1=rstd[:], op0=OP.mult, op1=OP.mult)
            xn = k.p_misc.tile([P, D], F32, tag="htm")
            nc.scalar.activation(xn[:], hh[:], AF.Identity, scale=rstd[:],
                                 bias=nmr[:])
            for c in range(NCH):
                pst = k.p_pst.tile([P, P], F32, tag="psrow", space="PSUM")
                nc.tensor.transpose(pst[:], xn[:, P * c:P * (c + 1)], k.ident[:])
                nc.scalar.activation(h_B[part][c][:, P * t:P * (t + 1)], pst[:],
                                     AF.Identity, scale=g_model[:, c:c + 1],
                                     bias=b_model[:, c:c + 1])

    _tap(k, taps, "dbg_hr", h_B["r"])
    _tap(k, taps, "dbg_hi", h_B["i"])

    # ---- attention layer ----
    def attn_layer(prefix, xs_r, xs_i, ntok, seqlen, spool):
        n_kch = seqlen // P              # ki chunks per sequence (2/1/1)
        nsb = seqlen // P

        (rstd_br, nmr_br), ri = k.ln_stats2_B(xs_r, xs_i, ntok)
        xn_r = k.ln_apply_B(xs_r, rstd_br, nmr_br, ntok)
        xn_i = k.ln_apply_B(xs_i, ri[0], ri[1], ntok)

        wq = k.load_w(din[f"{prefix}_qT"])
        wk = k.load_w(din[f"{prefix}_kT"])
        wv = k.load_w(din[f"{prefix}_vT"])
        wo = k.load_w(din[f"{prefix}_oT"])
        qb = k.p_w.tile([P, H], F32, tag="hb", bufs=6)
        nc.sync.dma_start(qb[:], din[f"{prefix}_qb"][:])
        kb = k.p_w.tile([P, H], F32, tag="hb", bufs=6)
        nc.sync.dma_start(kb[:], din[f"{prefix}_kb"][:])
        ob = k.p_w.tile([P, NCH], F32, tag="hb", bufs=6)
        nc.sync.dma_start(ob[:], din[f"{prefix}_ob"][:])
        vb_row = k.p_row.tile([1, D], F32, tag="vbrow", bufs=2)
        nc.sync.dma_start(vb_row[:], din[f"{prefix}_vbrow"][:])
        vb_b = k.p_bc.tile([P, D], F32, tag="vbb", bufs=2)
        nc.gpsimd.partition_broadcast(vb_b[:], vb_row[:])

        def qk_cat(w, bias):
            tiles = []
            for h in range(H):
                ps = k.p_ps.tile([P, ntok], F32, tag="ps", space="PSUM")
                for c in range(NCH):
                    nc.tensor.matmul(
                        ps[0:64, :], k.wslice(w, c)[:, 64 * h:64 * h + 64],
                        xn_r[c][:], start=(c == 0), stop=(c == NCH - 1),
                        tile_position=(0, 0))
                for c in range(NCH):
                    nc.tensor.matmul(
                        ps[64:128, :], k.wslice(w, c)[:, 64 * h:64 * h + 64],
                        xn_i[c][:], start=(c == 0), stop=(c == NCH - 1),
                        tile_position=(0, 64))
                t = k.p_qkv.tile([P, ntok], BF16, tag="qkcat", bufs=18)
                nc.vector.tensor_scalar(out=t[:], in0=ps[:],
                                        scalar1=bias[:, h:h + 1], scalar2=None,
                                        op0=OP.add)
                tiles.append(t)
            return tiles

        q_cat = qk_cat(wq, qb)
        k_cat = qk_cat(wk, kb)

        vT = {"r": [], "i": []}
        for part, xn in (("r", xn_r), ("i", xn_i)):
            for m in range(ntok // P):
                ps = k.p_ps.tile([P, D], F32, tag="ps", space="PSUM")
                for c in range(NCH):
                    nc.tensor.matmul(ps[:], xn[c][:, P * m:P * (m + 1)],
                                     k.wslice(wv, c), start=(c == 0),
                                     stop=(c == NCH - 1))
                t = k.p_qkv.tile([P, D], BF16, tag="vt", bufs=8)
                nc.vector.tensor_tensor(out=t[:], in0=ps[:], in1=vb_b[:],
                                        op=OP.add)
                vT[part].append(t)

        attn_s = {"r": [], "i": []}
        for c in range(NCH):
            attn_s["r"].append(k.p_attn.tile([P, ntok], BF16, tag="attnr",
                                             bufs=5, name=f"attnr{c}"))
            attn_s["i"].append(k.p_attn.tile([P, ntok], BF16, tag="attni",
                                             bufs=5, name=f"attni{c}"))

        for b in range(BPC):
            sl = slice(seqlen * b, seqlen * (b + 1))
            for h in range(H):
                sums = k.p_psr.tile([1, seqlen], F32, tag="psrow", space="PSUM")
                expT = []
                for j in range(n_kch):
                    pss = k.p_ps.tile([P, seqlen], F32, tag="ps", space="PSUM")
                    nc.tensor.matmul(
                        pss[:],
                        k_cat[h][:, seqlen * b + P * j:seqlen * b + P * (j + 1)],
                        q_cat[h][:, sl], start=True, stop=True)
                    e = k.p_attn.tile([P, seqlen], BF16, tag="expT", bufs=6)
                    nc.scalar.activation(e[:], pss[:], AF.Exp, scale=1.0 / 8.0)
                    expT.append(e)
                    nc.tensor.matmul(sums[:], k.ones16[:, 0:1], e[:],
                                     start=(j == 0), stop=(j == n_kch - 1))
                rr = k.p_misc.tile([1, seqlen], F32, tag="rowpack", bufs=4)
                nc.vector.reciprocal_approx_fast(rr[:], sums[:])
                rb = k.p_attn.tile([P, seqlen], F32, tag="rb", bufs=3)
                nc.gpsimd.partition_broadcast(rb[:], rr[:])

                ps = k.p_ps.tile([P, seqlen], F32, tag="ps", space="PSUM")
                for part, cofs in (("r", 0), ("i", 64)):
                    for j in range(n_kch):
                        nc.tensor.matmul(
                            ps[cofs:cofs + 64, :],
                            vT[part][b * nsb + j][:, 64 * h:64 * h + 64],
                            expT[j][:], start=(j == 0), stop=(j == n_kch - 1),
                            tile_position=(0, cofs))
                cp, half = h // 2, h % 2
                nc.vector.tensor_tensor(
                    out=attn_s["r"][cp][64 * half:64 * half + 64, sl],
                    in0=ps[0:64, :], in1=rb[0:64, :], op=OP.mult)
                nc.vector.tensor_tensor(
                    out=attn_s["i"][cp][64 * half:64 * half + 64, sl],
                    in0=ps[64:128, :], in1=rb[64:128, :], op=OP.mult)

        new_r, new_i = [], []
        for part, attn_t, xs, outl in (("r", attn_s["r"], xs_r, new_r),
                                       ("i", attn_s["i"], xs_i, new_i)):
            for m in range(NCH):
                ps = k.p_ps.tile([P, ntok], F32, tag="ps", space="PSUM")
                for c in range(NCH):
                    nc.tensor.matmul(ps[:], k.wslice(wo, c, m), attn_t[c][:],
                                     start=(c == 0), stop=(c == NCH - 1))
                o = spool.tile([P, ntok], F32R, tag=f"st{ntok}{part}", bufs=8,
                               name=f"res{part}{m}")
                nc.vector.scalar_tensor_tensor(out=o[:], in0=ps[:],
                                               scalar=ob[:, m:m + 1],
                                               in1=xs[m][:], op0=OP.add,
                                               op1=OP.add)
                outl.append(o)
        return new_r, new_i

    def pool2(xs, ntok, dst_pool, part):
        outs = []
        for c in range(NCH):
            o = dst_pool.tile([P, ntok // 2], F32, tag=f"st{ntok // 2}{part}")
            nc.vector.tensor_tensor(out=o[:], in0=xs[c][:, 0:ntok:2],
                                    in1=xs[c][:, 1:ntok:2], op=OP.add)
            nc.vector.tensor_scalar(out=o[:], in0=o[:], scalar1=0.5,
                                    scalar2=None, op0=OP.mult)
            outs.append(o)
        return outs

    def proj_up(prefix, src_r, src_i, base_r, base_i, ntok_half, dst_pool):
        """base + clin_rb(up2(src), proj): bias on real part only."""
        w = k.load_w(din[f"{prefix}T"])
        bsl = k.p_w.tile([P, NCH], F32, tag="hb")
        nc.sync.dma_start(bsl[:], din[f"{prefix}b"][:])
        ntok = ntok_half * 2
        outs = {"r": [], "i": []}
        for part, src, base in (("r", src_r, base_r), ("i", src_i, base_i)):
            src16 = []
            for c in range(NCH):
                t = k.p_xn.tile([P, ntok_half], BF16, tag="xn")
                nc.vector.tensor_copy(t[:], src[c][:])
                src16.append(t)
            for m in range(NCH):
                ps = k.p_ps.tile([P, ntok], F32, tag="ps", space="PSUM")
                for c in range(NCH):
                    rep = src16[c][:].rearrange("p (n o) -> p n o", o=1) \
                        .broadcast_to([P, ntok_half, 2])
                    nc.tensor.matmul(ps[:], k.wslice(w, c, m), rep,
                                     start=(c == 0), stop=(c == NCH - 1))
                o = dst_pool.tile([P, ntok], F32, tag=f"st{ntok}{part}")
                if part == "r":
                    nc.vector.scalar_tensor_tensor(out=o[:], in0=ps[:],
                                                   scalar=bsl[:, m:m + 1],
                                                   in1=base[m][:], op0=OP.add,
                                                   op1=OP.add)
                else:
                    nc.vector.tensor_tensor(out=o[:], in0=ps[:], in1=base[m][:],
                                            op=OP.add)
                outs[part].append(o)
        return outs["r"], outs["i"]

    phr_r = pool2(word_r, TOK, k.p_s256, "r")
    phr_i = pool2(word_i, TOK, k.p_s256, "i")
    for i in range(DEPTH):
        phr_r, phr_i = attn_layer(f"p{i}", phr_r, phr_i, TOK // 2, S // 2,
                                  k.p_s256)

    comb_r, comb_i = proj_up("wp", phr_r, phr_i, word_r, word_i, TOK // 2,
                             k.p_s512)

    sent_r = pool2(comb_r, TOK, k.p_s256, "r")
    sent_i = pool2(comb_i, TOK, k.p_s256, "i")
    for i in range(max(1, DEPTH // 2)):
        sent_r, sent_i = attn_layer(f"s{i}", sent_r, sent_i, TOK // 2, S // 2,
                                    k.p_s256)

    pre_r, pre_i = proj_up("ps", sent_r, sent_i, comb_r, comb_i, TOK // 2,
                           k.p_s512)

    # ---- fused cln (full apply with ln_hier g/b) ----
    rstd_b, nmr_b = k.ln_stats_B(pre_r, TOK)
    fused_r = k.ln_apply_B(pre_r, rstd_b, nmr_b, TOK, out_dtype=F32,
                           out_pool=k.p_f32a, out_tag="fusedr",
                           gb=(g_hier, b_hier))
    rstd_b, nmr_b = k.ln_stats_B(pre_i, TOK)
    fused_i = k.ln_apply_B(pre_i, rstd_b, nmr_b, TOK, out_dtype=F32,
                           out_pool=k.p_f32a, out_tag="fusedi",
                           gb=(g_hier, b_hier))
    _tap(k, taps, "dbg_fusedr", fused_r)
    _tap(k, taps, "dbg_fusedi", fused_i)

    # ---- memory read ----
    fused_r16 = []
    for c in range(NCH):
        t = k.p_xn.tile([P, TOK], BF16, tag="xn")
        nc.vector.tensor_copy(t[:], fused_r[c][:])
        fused_r16.append(t)

    w_memT = k.load_w(din["memT"])
    w_memr = k.load_w(din["mem_r"])
    w_memi = k.load_w(din["mem_i"])

    nsq = []
    for c in range(NCH):
        s1 = k.p_misc.tile([P, TOK], F32, tag="sq")
        nc.scalar.activation(s1[:], fused_r[c][:], AF.Square)
        s2 = k.p_misc.tile([P, TOK], F32, tag="sq")
        nc.scalar.activation(s2[:], fused_i[c][:], AF.Square)
        t = k.p_misc.tile([P, TOK], F32, tag="nsq")
        nc.vector.tensor_tensor(out=t[:], in0=s1[:], in1=s2[:], op=OP.add)
        nsq.append(t)

    wT_mem = []
    for cm in range(NCH):
        wT_mem.append(k.p_qkv.tile([P, TOK], BF16, tag="wTm", name=f"wTm{cm}"))

    for m in range(TOK // P):
        ps_sc = k.p_ps.tile([P, M_MEM], F32, tag="ps", space="PSUM")
        for c in range(NCH):
            nc.tensor.matmul(ps_sc[:], fused_r16[c][:, P * m:P * (m + 1)],
                             k.wslice(w_memT, c), start=(c == 0),
                             stop=(c == NCH - 1))
        nT = k.p_misc.tile([P, D], F32, tag="nT")
        for c in range(NCH):
            pst = k.p_pst.tile([P, P], F32, tag="psrow", space="PSUM")
            nc.tensor.transpose(pst[:], nsq[c][:, P * m:P * (m + 1)], k.ident[:])
            nc.scalar.activation(nT[:, P * c:P * (c + 1)], pst[:], AF.Ln)
        nrm = k.p_misc.tile([P, D], F32, tag="nT")
        nc.scalar.activation(nrm[:], nT[:], AF.Exp, scale=0.5)
        nc.vector.tensor_scalar(out=nrm[:], in0=nrm[:], scalar1=1e-8,
                                scalar2=None, op0=OP.add)
        rn = k.p_misc.tile([P, D], F32, tag="nT")
        nc.vector.reciprocal_approx_fast(rn[:], nrm[:])
        z = k.p_misc.tile([P, M_MEM], F32, tag="nT")
        nc.vector.tensor_tensor(out=z[:], in0=ps_sc[:], in1=rn[:], op=OP.mult)
        negmax = k.p_row.tile([P, 1], F32, tag="cst")
        nc.vector.reduce_max(negmax[:], z[:], AX.X, negate=True)
        ez = k.p_misc.tile([P, M_MEM], BF16, tag="ez")
        ssum = k.p_row.tile([P, 1], F32, tag="cst")
        nc.scalar.activation(ez[:], z[:], AF.Exp, bias=negmax[:],
                             accum_out=ssum[:])
        rs = k.p_row.tile([P, 1], F32, tag="cst")
        nc.vector.reciprocal_approx_fast(rs[:], ssum[:])
        wgt = k.p_misc.tile([P, M_MEM], BF16, tag="ez")
        nc.vector.tensor_scalar(out=wgt[:], in0=ez[:], scalar1=rs[:],
                                scalar2=None, op0=OP.mult)
        for cm in range(NCH):
            pst16 = k.p_pst.tile([P, P], BF16, tag="psrow", space="PSUM")
            nc.tensor.transpose(pst16[:], wgt[:, P * cm:P * (cm + 1)],
                                k.ident16[:])
            nc.vector.tensor_copy(wT_mem[cm][:, P * m:P * (m + 1)], pst16[:])

    rd_w = k.load_w(din["rdT"])
    rd_b = k.p_w.tile([P, NCH], F32, tag="hb")
    nc.sync.dma_start(rd_b[:], din["rdb"][:])

    h2 = {"r": [], "i": []}
    for part, wmem, fus in (("r", w_memr, fused_r), ("i", w_memi, fused_i)):
        cont16 = []
        for cd in range(NCH):
            ps = k.p_ps.tile([P, TOK], F32, tag="ps", space="PSUM")
            for cm in range(NCH):
                nc.tensor.matmul(ps[:], k.wslice(wmem, cm, cd), wT_mem[cm][:],
                                 start=(cm == 0), stop=(cm == NCH - 1))
            t = k.p_xn.tile([P, TOK], BF16, tag="xn")
            nc.vector.tensor_copy(t[:], ps[:])
            cont16.append(t)
        for m in range(NCH):
            ps = k.p_ps.tile([P, TOK], F32, tag="ps", space="PSUM")
            for c in range(NCH):
                nc.tensor.matmul(ps[:], k.wslice(rd_w, c, m), cont16[c][:],
                                 start=(c == 0), stop=(c == NCH - 1))
            o = k.p_f32a.tile([P, TOK], F32, tag=f"h2{part}")
            nc.vector.scalar_tensor_tensor(out=o[:], in0=ps[:],
                                           scalar=rd_b[:, m:m + 1],
                                           in1=fus[m][:], op0=OP.add, op1=OP.add)
            h2[part].append(o)

    # ---- final cln + density ----
    rstd_b, nmr_b = k.ln_stats_B(h2["r"], TOK)
    hn_r = k.ln_apply_B(h2["r"], rstd_b, nmr_b, TOK, out_dtype=F32,
                        out_pool=k.p_f32a, out_tag="hn")
    rstd_b, nmr_b = k.ln_stats_B(h2["i"], TOK)
    hn_i = k.ln_apply_B(h2["i"], rstd_b, nmr_b, TOK, out_dtype=F32,
                        out_pool=k.p_f32a, out_tag="hn")
    density16 = []
    for c in range(NCH):
        d1 = k.p_misc.tile([P, TOK], F32, tag="sq")
        nc.scalar.activation(d1[:], hn_r[c][:], AF.Square,
                             scale=g_model[:, c:c + 1], bias=b_model[:, c:c + 1])
        d2 = k.p_misc.tile([P, TOK], F32, tag="sq")
        nc.scalar.activation(d2[:], hn_i[c][:], AF.Square,
                             scale=g_model[:, c:c + 1], bias=b_model[:, c:c + 1])
        dt_ = k.p_misc.tile([P, TOK], BF16, tag="dens")
        nc.vector.tensor_tensor(out=dt_[:], in0=d1[:], in1=d2[:], op=OP.add)
        density16.append(dt_)
    if "dbg_density" in taps:
        for c in range(NCH):
            f32t = k.p_misc.tile([P, TOK], F32, tag="sq")
            nc.vector.tensor_copy(f32t[:], density16[c][:])
            nc.sync.dma_start(taps["dbg_density"][P * c:P * (c + 1), :], f32t[:])

    # ---- collapse ----
    cb_t = k.p_const.tile([P, NVT], F32, tag="cb")
    nc.sync.dma_start(cb_t[:], din["cb"][:])
    for t in range(NVT):
        cw = k.p_w.tile([P, D], BF16, tag="cw")
        nc.sync.dma_start(cw[:], din["cwT"][:, t, :])
        ps = k.p_ps.tile([P, TOK], F32, tag="ps", space="PSUM")
        for c in range(NCH):
            nc.tensor.matmul(ps[:], cw[:, P * c:P * (c + 1)], density16[c][:],
                             start=(c == 0), stop=(c == NCH - 1))
        o = k.p_out.tile([P, TOK], BF16, tag="out")
        nc.scalar.activation(o[:], ps[:], AF.Relu, bias=cb_t[:, t:t + 1])
        nc.sync.dma_start(out_dram[P * t:P * (t + 1), :], o[:])


# ----------------------------------------------------------------------------
# entry point
# ----------------------------------------------------------------------------

_RUN_KW = {}


def kernel(x=None, params=None, **kw):
    if x is None:
        x = kw.pop("x")
    if params is None:
        params = kw.pop("params")
    in_maps = _prep(x, params)
    nc = build_nc(debug=DEBUG)
    res = run_bass_kernel_spmd(nc, in_maps, core_ids=list(range(NCORES)),
                               **_RUN_KW)
    outs = []
    for c in range(NCORES):
        o = np.asarray(res.results[c]["out"])          # [V, TOK] bf16
        o = o.reshape(V, BPC, S).transpose(1, 2, 0).astype(np.float32)
        outs.append(o)
    full = np.concatenate(outs, axis=0)                # [B, S, V]
    kernel.last_results = res
    return full
